# revision 102
# baseline (speedup 1.0000x reference)
"""DRNL filterbank Trainium2 kernel, v5.

Banded-Toeplitz formulation (ME/LIN/AFT bf16 band matmuls, BEF fp8
DoubleRow pairs, broken-stick pointwise on vector+scalar).

v5: 3-section chunks. A matmul's psum output must fit one 2KB bank
(512 f32) and the 3-section width 2*SEC+R = 491 fits exactly, so each
LIN/BEF/AFT band runs once per 3 batches instead of once per 2
(28 -> 21 chunks, 264 -> 198 matmuls/core) while keeping the original
1-bank psum rotation (o x4, v x3, me). 4-section (658) fails the ISA
check. The ~140ns fixed per-matmul cost makes fewer-but-wider strictly
better. ME also runs in 3-batch groups aligned with the chunk groups.
Startup: each DMA costs ~1.3us of DMA-engine pipeline latency, so the
whole first-matmul critical path (wme + ME group 0's xf slice) ships
as ONE "head" DMA on the SP queue; the Act queue streams blob8/blobB
in parallel on its own DMA engine. Do NOT shrink the work pool below
bufs=4 (bufs=2 miscomputes), do NOT truncate the ME FIR (1e-3 trunc
-> 1.7e-2 rel err; per-channel LIN gains amplify it), and do NOT put
sqrt-mode on the light tail chunks (serial 3-act latency stalls AFT).

v4 vs v3:
  - DMA consolidation: weight blobs packed in first-use order (bf16
    LIN/AFT blobs + fp8 BEF blob, blobB split in two), one contiguous
    xf DMA (k-major host fold), host-replicated scal, quad output DMAs.
    54 -> 14 DMA_DIRECT2D issues across both HWDGE queues (SP carries
    wme/xf/scal + outputs, Activation carries the weight blobs so the
    first-matmul path skips the ACT_TABLE_LOAD stall).
  - Truncation tolerances raised to 4e-3 / 8e-2 (292 -> 264 matmuls):
    the output error is floored by the final bf16 output-copy
    quantization at the response peak (~4e-3 rel), so shorter IR tails
    are free until truncation error approaches that floor.
  - fp8 DoubleRow for AFT was tried and reverted: pair matmuls measure
    ~450ns vs ~280ns bf16 singles (DoubleRow disables FWL and the dual
    rhs stream is fetch-bound), so the win never materializes, and the
    required one-column-shifted fp8 w copy stalls the PE. The PE also
    rejects small pair-dim rhs strides at runtime (dup_off=-1 aborts),
    so a second shifted signal copy is mandatory for any pair rhs.
Sharding: channels across 8 cores, slot-structured SPMD.
"""
import numpy as np
import ml_dtypes

P = 128
B, T, F = 8, 20000, 50
R = (T + P - 1) // P
N_CORES = 8
N_SLOTS = 7
TRUNC_LIN = 4e-3
TRUNC_NL = 8e-2
IR_LEN = 4096
BME = 5
PADX = 4

_CACHE = {}

BF16 = ml_dtypes.bfloat16
FP8 = ml_dtypes.float8_e4m3fn

# 3-section chunks: a matmul's psum output must stay within one 2KB bank
# (512 f32), and 2*SEC+WW = 491 fits exactly -- so each chunk covers 3
# batches (the last group covers the 2 remaining). 21 chunks total.
CHUNK_SLOTS = [4, 0, 6, 1, 5, 2, 3]
_GROUP_ORDERS = {0: [4, 0, 6, 1, 5, 2, 3], 1: [0, 6, 1, 5, 2, 4, 3],
                 2: [0, 6, 1, 5, 2, 3, 4]}
CHUNKS = [(s, h) for h in (0, 1, 2) for s in _GROUP_ORDERS[h]]
NCH = len(CHUNKS)
NSEC_H = [3, 3, 2]   # batches (sections groups) per h
QH = [0, 3, 6]       # first batch of each group
# pointwise engine schedule (tuned against the trace):
C_MODE = ["int" if (j * 14) % 21 < 14 else "sqrt" for j in range(NCH)]
MT_ENG = ["vector"] * NCH           # STT min(a*v, c)   (PSUM read -> vector only)
WT_ENG = ["vector"] * NCH           # STT w = max(-c, m) (Pool rejects STT entirely)
OC_ENG = ["scalar"] * NCH           # psum -> bf16 output copy
XC_ENG = ["gpsimd", "gpsimd"]       # xme8 main/shifted cast engines
BEF_LAG = 1   # BEF_j issues one chunk-slot after LIN_j
AFT_LAG = 3   # AFT_j issues in iteration j+AFT_LAG
BLOBA_SLOTS = CHUNK_SLOTS[:2]
# blob B packed in first-use order (LIN at slot's first chunk, AFT 3 chunks
# later), split into two DMAs after the 5th block so early consumers don't
# wait on the full transfer
BLOBB_ORDER = [("lin", 6), ("lin", 1), ("aft", 4), ("lin", 5), ("aft", 0),
               ("lin", 2), ("aft", 6), ("lin", 3), ("aft", 1), ("aft", 5),
               ("aft", 2), ("aft", 3)]
BLOBB_SPLIT = 5


def _lfilter_vec(x, b, a):
    b0, b1, b2 = b[:, 0], b[:, 1], b[:, 2]
    a1, a2 = a[:, 1], a[:, 2]
    y = np.zeros_like(x)
    z1 = np.zeros(x.shape[0])
    z2 = np.zeros(x.shape[0])
    for t in range(x.shape[-1]):
        xt = x[:, t]
        yt = b0 * xt + z1
        z1 = b1 * xt - a1 * yt + z2
        z2 = b2 * xt - a2 * yt
        y[:, t] = yt
    return y


def _cascade_ir(b, a, n, times):
    h = np.zeros((b.shape[0], n))
    h[:, 0] = 1.0
    for _ in range(times):
        h = _lfilter_vec(h, b, a)
    return h


def _trunc(h, tol):
    m = np.abs(h).max()
    idx = np.nonzero(np.abs(h) > tol * m)[0]
    return h[: int(idx[-1]) + 1] if len(idx) else h[:1]


def _nb(h):
    return (len(h) + P - 2) // P + 1


def _toeplitz_band(h, d):
    k = np.arange(P)[:, None]
    i = np.arange(P)[None, :]
    idx = P * d + i - k
    ok = (idx >= 0) & (idx < len(h))
    return np.where(ok, np.asarray(h, np.float64)[np.clip(idx, 0, len(h) - 1)], 0.0)


def _bands_cat(h, nb, dtype):
    W = np.concatenate([_toeplitz_band(h, d) for d in range(nb)], axis=1)
    return np.ascontiguousarray(W).astype(dtype)


def _pairs_cat(h, nbp, dtype):
    mats = []
    for p in range(nbp):
        mats.append(_toeplitz_band(h, 2 * p))
        mats.append(_toeplitz_band(h, 2 * p + 1))
    W = np.concatenate(mats, axis=1)
    return np.ascontiguousarray(W).astype(dtype)


def _tune_k(b):
    """Best int32 bias K: bitcast((bits(x)>>2)+K) ~= b*x**0.25 over x range."""
    x = np.float32(np.logspace(-6, 1.5, 4000))
    t1 = (x.view(np.int32) & 0x7FFFFFFF) >> 2
    ref = np.float64(b) * np.float64(x) ** 0.25
    b0 = 127 << 23
    base = np.int64(np.float32(b).view(np.int32)) - b0 // 4
    best = (1e9, 0)
    for sig in np.linspace(-0.02, 0.10, 121):
        K = np.int64(base - int(sig * (1 << 23)))
        c = (t1.astype(np.int64) + K).astype(np.int32).view(np.float32)
        e = np.abs(c - ref) / ref
        best = min(best, (float(e.max()), int(K)))
    return best[1]


def _build_host(me_fir, lin_fir, nlin_fir_before, nlin_fir_after,
                lpf_lin_b, lpf_lin_a, lpf_nlin_b, lpf_nlin_a,
                lin_gain, nlin_a, nlin_b):
    ir4 = _cascade_ir(lpf_lin_b.astype(np.float64), lpf_lin_a.astype(np.float64), IR_LEN, 4)
    ir3 = _cascade_ir(lpf_nlin_b.astype(np.float64), lpf_nlin_a.astype(np.float64), IR_LEN, 3)

    scale = 10.0 ** ((93.98 - 100.0) / 20.0)
    ME = np.asarray(me_fir, np.float64) * scale
    bme = _nb(ME)

    LIN, BEF, AFT = [], [], []
    for f in range(F):
        LIN.append(_trunc(lin_gain[f] * np.convolve(np.asarray(lin_fir[f], np.float64), ir4[f]), TRUNC_LIN))
        BEF.append(_trunc(np.asarray(nlin_fir_before[f], np.float64), TRUNC_NL))
        AFT.append(_trunc(np.convolve(np.asarray(nlin_fir_after[f], np.float64), ir3[f]), TRUNC_NL))

    nbp = lambda h: (_nb(h) + 1) // 2
    cost = [_nb(LIN[f]) + nbp(BEF[f]) + _nb(AFT[f]) for f in range(F)]
    order = np.argsort(-np.asarray(cost), kind="stable")

    slot_ch = np.zeros((N_CORES, N_SLOTS), np.int64)
    for s in range(6):
        for c in range(N_CORES):
            slot_ch[c, s] = order[8 * s + c]
    for c in range(N_CORES):
        slot_ch[c, 6] = order[48 + (c % 2)]

    BL = [max(_nb(LIN[slot_ch[c, s]]) for c in range(N_CORES)) for s in range(N_SLOTS)]
    BBp = [max(nbp(BEF[slot_ch[c, s]]) for c in range(N_CORES)) for s in range(N_SLOTS)]
    BA = [max(_nb(AFT[slot_ch[c, s]]) for c in range(N_CORES)) for s in range(N_SLOTS)]

    sh = max(max(BL) - 1, 2 * max(BBp) - 1, max(BA) - 1)
    PADS = (sh + 1 + 1) // 2 * 2

    gam = np.ones((N_CORES, N_SLOTS), np.float64)

    wme = _bands_cat(ME, bme, BF16)
    blobA, blobB, blob8 = [], [], []
    for c in range(N_CORES):
        a16 = [_bands_cat(gam[c, s] * LIN[slot_ch[c, s]], BL[s], BF16)
               for s in BLOBA_SLOTS]
        b16 = []
        for kind, s in BLOBB_ORDER:
            h = (gam[c, s] * LIN[slot_ch[c, s]] if kind == "lin"
                 else gam[c, s] * AFT[slot_ch[c, s]])
            b16.append(_bands_cat(h, BL[s] if kind == "lin" else BA[s], BF16))
        p8 = [_pairs_cat(BEF[slot_ch[c, s]], BBp[s], FP8) for s in CHUNK_SLOTS]
        blobA.append(np.concatenate(a16, axis=1))
        blobB.append(np.concatenate(b16, axis=1))
        blob8.append(np.concatenate(p8, axis=1))

    scal = np.zeros((N_CORES, N_SLOTS * 4), np.float32)
    for c in range(N_CORES):
        for s in range(N_SLOTS):
            f = slot_ch[c, s]
            scal[c, 4 * s + 0] = nlin_a[f]
            scal[c, 4 * s + 1] = float(nlin_b[f]) ** 4
            scal[c].view(np.int32)[4 * s + 2] = _tune_k(float(nlin_b[f]))

    return {
        "slot_ch": slot_ch, "BL": BL, "BBp": BBp, "BA": BA, "PADS": PADS,
        "gam": gam, "BME": bme,
        "wme": wme, "blobA": blobA, "blobB": blobB, "blob8": blob8,
        "scal": scal,
    }


def _fold_x(x):
    # k-major layout [P, B, XSEC]: the xf DMA is 128 contiguous rows
    xp = np.zeros((B, R * P), np.float32)
    xp[:, :T] = x
    xf = np.zeros((P, B, PADX + R), np.float32)
    xf[:, :, PADX:] = xp.reshape(B, R, P).transpose(2, 0, 1)
    return np.ascontiguousarray(xf).astype(BF16)


def _build_program(meta):
    import concourse.bacc as bacc
    import concourse.bass as bass
    from concourse import mybir
    from concourse.tile import TileContext

    BL, BBp, BA, PADS = meta["BL"], meta["BBp"], meta["BA"], meta["PADS"]
    BME = meta["BME"]
    SEC = PADS + R
    WW = SEC + R
    W3 = 2 * SEC + R     # 3-section chunk width (491 f32: fits one psum bank)
    WH = [W3, W3, WW]    # matmul/pointwise width per group h
    XSEC = PADX + R
    XW = XSEC + R
    f32, bf16, f8 = mybir.dt.float32, mybir.dt.bfloat16, mybir.dt.float8e4
    i32 = mybir.dt.int32
    AF = mybir.ActivationFunctionType
    ALU = mybir.AluOpType
    DR = mybir.MatmulPerfMode.DoubleRow

    olA, olB, ob8, oa = {}, {}, {}, {}
    off = 0
    for s in BLOBA_SLOTS:
        olA[s] = off
        off += BL[s]
    nA = off
    off = 0
    nB_split = 0
    for bi, (kind, s) in enumerate(BLOBB_ORDER):
        if kind == "lin":
            olB[s] = off
            off += BL[s]
        else:
            oa[s] = off
            off += BA[s]
        if bi + 1 == BLOBB_SPLIT:
            nB_split = off
    nB = off
    off = 0
    for s in CHUNK_SLOTS:
        ob8[s] = off
        off += BBp[s]
    n8 = off

    HEADW = BME * P + 3 * XSEC
    nc = bacc.Bacc("TRN2", target_bir_lowering=False, debug=False, num_devices=N_CORES)
    # head = wme bands + ME group 0's xf slice: the whole first-matmul
    # critical path arrives with a single DMA (each extra DMA adds ~1.3us
    # of DMA-engine pipeline latency)
    d_head = nc.dram_tensor("head", [P, HEADW], bf16, kind="ExternalInput").ap()
    d_xfr = nc.dram_tensor("xfr", [P, 5 * XSEC], bf16, kind="ExternalInput").ap()
    d_blobA = nc.dram_tensor("blobA", [P, nA * P], bf16, kind="ExternalInput").ap()
    d_blobB = nc.dram_tensor("blobB", [P, nB * P], bf16, kind="ExternalInput").ap()
    d_blob8 = nc.dram_tensor("blob8", [P, n8 * 2 * P], f8, kind="ExternalInput").ap()
    d_scal = nc.dram_tensor("scal", [P, N_SLOTS * 4], f32, kind="ExternalInput").ap()
    d_out = nc.dram_tensor("yout", [7, P, 3 * W3], bf16, kind="ExternalOutput").ap()

    def pair_rhs(tile, col_off, dup_off):
        base = tile[:, 0:1]
        return bass.AP(tensor=base.tensor, offset=base.offset + col_off,
                       ap=[[base.ap[0][0], P], [dup_off, 2], [1, WW]])

    def pair_lhs(tile, band_off, p):
        s = tile[:, (band_off + p) * 2 * P:(band_off + p + 1) * 2 * P]
        return bass.AP(tensor=s.tensor, offset=s.offset,
                       ap=[[s.ap[0][0], P], [P, 2], [1, P]])

    ENG = None  # set inside context

    with TileContext(nc) as tc:
        with (
            tc.tile_pool(name="singles", bufs=1) as singles,
            tc.tile_pool(name="work", bufs=4) as work,
            tc.tile_pool(name="ps", bufs=1, space="PSUM") as ps,
        ):
            ENG = {"vector": nc.vector, "scalar": nc.scalar, "gpsimd": nc.gpsimd}

            def ecopy(eng, out, in_):
                if eng == "scalar":
                    nc.scalar.activation(out, in_, AF.Copy)
                else:
                    ENG[eng].tensor_copy(out=out, in_=in_)
            # SP: head (first-matmul critical path), blobA (LIN chunk 0),
            # xf rest. Act: scal, then the remaining weight blobs — the two
            # queues transfer on separate DMA engines.
            head_t = singles.tile([P, HEADW], bf16)
            nc.sync.dma_start(out=head_t, in_=d_head)
            blobA_t = singles.tile([P, nA * P], bf16)
            nc.sync.dma_start(out=blobA_t, in_=d_blobA)
            xfr_t = singles.tile([P, 5 * XSEC], bf16)
            nc.sync.dma_start(out=xfr_t, in_=d_xfr)
            blob8_t = singles.tile([P, n8 * 2 * P], f8)
            nc.scalar.dma_start(out=blob8_t, in_=d_blob8)
            scal_t = singles.tile([P, N_SLOTS * 4], f32)
            nc.scalar.dma_start(out=scal_t, in_=d_scal)
            blobB_t = singles.tile([P, nB * P], bf16)
            nc.scalar.dma_start(out=blobB_t[:, :nB_split * P],
                                in_=d_blobB[:, :nB_split * P])
            nc.scalar.dma_start(out=blobB_t[:, nB_split * P:],
                                in_=d_blobB[:, nB_split * P:])

            def wl_slice(s, d):
                if s in olA:
                    return blobA_t[:, (olA[s] + d) * P:(olA[s] + d + 1) * P]
                return blobB_t[:, (olB[s] + d) * P:(olB[s] + d + 1) * P]

            def wa_slice(s, d):
                return blobB_t[:, (oa[s] + d) * P:(oa[s] + d + 1) * P]

            O8 = (B * SEC + 7) // 4 * 4
            xme16 = singles.tile([P, B * SEC], bf16)
            xme8 = singles.tile([P, O8 + B * SEC + 4], f8)
            for qq in range(B):
                nc.vector.memset(xme16[:, qq * SEC:qq * SEC + PADS], 0.0)
                nc.gpsimd.memset(xme8[:, qq * SEC:qq * SEC + PADS], 0.0)
                lo = O8 + qq * SEC + (1 if qq else 0)
                nc.gpsimd.memset(xme8[:, lo:O8 + qq * SEC + PADS + 1], 0.0)

            # ME in 3-batch groups aligned with the chunk groups: group h
            # produces exactly the xme sections chunk group h consumes
            XW3 = 2 * XSEC + R

            def emit_me(g):
                q = QH[g]
                n = NSEC_H[g]
                xw = (n - 1) * XSEC + R
                mp = ps.tile([P, XW3], f32, tag="me")
                for d in range(BME):
                    if g == 0:
                        ws = BME * P + q * XSEC + PADX - d
                        rhs = head_t[:, ws:ws + xw]
                    else:
                        ws = (q - 3) * XSEC + PADX - d
                        rhs = xfr_t[:, ws:ws + xw]
                    nc.tensor.matmul(mp[:, 0:xw], head_t[:, d * P:(d + 1) * P],
                                     rhs, start=(d == 0), stop=(d == BME - 1))
                for bi in range(n):
                    qq = q + bi
                    # spread the psum->xme16 copies across vector+scalar so
                    # the chunk's LIN (which needs all n sections) starts
                    # ~0.4us sooner than with a serial vector chain
                    ecopy("scalar" if bi == 1 else "vector",
                          xme16[:, qq * SEC + PADS:(qq + 1) * SEC],
                          mp[:, bi * XSEC:bi * XSEC + R])
                    src = xme16[:, qq * SEC + PADS:(qq + 1) * SEC]
                    ecopy(XC_ENG[0], xme8[:, qq * SEC + PADS:(qq + 1) * SEC], src)
                    ecopy(XC_ENG[1],
                          xme8[:, O8 + qq * SEC + PADS + 1:O8 + (qq + 1) * SEC + 1],
                          src)

            emit_me(0)

            o_psd, v_psd, w_td = {}, {}, {}
            w_bufs = []
            for i in range(AFT_LAG + 1):
                wbuf = singles.tile([P, 3 * SEC], bf16, tag=f"wbuf{i}")
                nc.vector.memset(wbuf[:, 0:PADS], 0.0)
                w_bufs.append(wbuf)
            oq_tiles = []
            for i in range(2):
                oq_t = singles.tile([P, 3 * W3], bf16, tag=f"oq{i}")
                oq_tiles.append(oq_t)

            def emit_lin(j):
                s, h = CHUNKS[j]
                q = QH[h]
                o_ps = ps.tile([P, W3], f32, tag=f"o{j % 4}")
                o_psd[j] = o_ps
                for d in range(BL[s]):
                    ws = q * SEC + PADS - d
                    nc.tensor.matmul(o_ps[:, 0:WH[h]], wl_slice(s, d),
                                     xme16[:, ws:ws + WH[h]],
                                     start=(d == 0), stop=False)

            def emit_bef_pointwise(j):
                s, h = CHUNKS[j]
                q = QH[h]
                wd = WH[h]
                a_ap = scal_t[:, 4 * s + 0:4 * s + 1]
                b4_ap = scal_t[:, 4 * s + 1:4 * s + 2]
                k_ap = scal_t.bitcast(i32)[:, 4 * s + 2:4 * s + 3]
                v_ps = ps.tile([P, W3], f32, tag=f"v{j % 3}")
                v_psd[j] = v_ps
                for p in range(BBp[s]):
                    base = xme8[:, 0:1]
                    rhs = bass.AP(tensor=base.tensor,
                                  offset=base.offset + q * SEC + PADS - 2 * p,
                                  ap=[[base.ap[0][0], P], [O8, 2], [1, wd]])
                    nc.tensor.matmul(v_ps[:, 0:wd], pair_lhs(blob8_t, ob8[s], p),
                                     rhs,
                                     start=(p == 0), stop=(p == BBp[s] - 1),
                                     perf_mode=DR)
                c_t = work.tile([P, W3], f32, tag="c")
                if C_MODE[j] == "sqrt":
                    u_t = work.tile([P, W3], f32, tag="u")
                    nc.scalar.activation(u_t[:, 0:wd], v_ps[:, 0:wd], AF.Abs)
                    nc.scalar.activation(c_t[:, 0:wd], u_t[:, 0:wd], AF.Sqrt,
                                         scale=b4_ap)
                    nc.scalar.sqrt(c_t[:, 0:wd], c_t[:, 0:wd])
                else:
                    t1 = work.tile([P, W3], i32, tag="u")
                    nc.vector.tensor_scalar(
                        out=t1[:, 0:wd], in0=v_ps[:, 0:wd].bitcast(i32),
                        scalar1=0x7FFFFFFF, op0=ALU.bitwise_and,
                        scalar2=2, op1=ALU.logical_shift_right)
                    nc.scalar.activation(c_t[:, 0:wd].bitcast(i32), t1[:, 0:wd],
                                         AF.Identity, bias=k_ap)
                m_t = work.tile([P, W3], f32, tag="m")
                ENG[MT_ENG[j]].scalar_tensor_tensor(
                    out=m_t[:, 0:wd], in0=v_ps[:, 0:wd], scalar=a_ap,
                    in1=c_t[:, 0:wd], op0=ALU.mult, op1=ALU.min,
                )
                w_t = w_bufs[j % len(w_bufs)]
                w_td[j] = w_t
                ENG[WT_ENG[j]].scalar_tensor_tensor(
                    out=w_t[:, PADS:PADS + wd], in0=c_t[:, 0:wd], scalar=-1.0,
                    in1=m_t[:, 0:wd], op0=ALU.mult, op1=ALU.max,
                )
                # re-zero the mid pads the stt overwrote (1 or 2 of them),
                # on the same queue as the stt
                pm = w_t[:, SEC:SEC + 1]
                ENG[WT_ENG[j]].memset(
                    bass.AP(tensor=pm.tensor, offset=pm.offset,
                            ap=[[pm.ap[0][0], P], [SEC, NSEC_H[h] - 1],
                                [1, PADS]]), 0.0)

            def emit_aft(j):
                s, h = CHUNKS[j]
                wd = WH[h]
                o_ps, w_t = o_psd.pop(j), w_td.pop(j)
                for d in range(BA[s]):
                    ws = PADS - d
                    nc.tensor.matmul(o_ps[:, 0:wd], wa_slice(s, d),
                                     w_t[:, ws:ws + wd],
                                     start=False, stop=(d == BA[s] - 1))
                oq = oq_tiles[(j // 3) % 2]
                out_t = oq[:, (j % 3) * W3:(j % 3) * W3 + wd]
                ecopy(OC_ENG[j], out_t, o_ps[:, 0:wd])
                if j % 3 == 2 or j == NCH - 1:
                    nc.sync.dma_start(out=d_out[j // 3], in_=oq)

            for t in range(NCH + AFT_LAG):
                if t == 2:
                    emit_me(1)
                if t == 8:
                    emit_me(2)
                # BEF before LIN: the pointwise chain for chunk t-1 gets a
                # LIN-duration head start, so w is ready before AFT needs it
                if 0 <= t - BEF_LAG < NCH:
                    emit_bef_pointwise(t - BEF_LAG)
                if t < NCH:
                    emit_lin(t)
                if 0 <= t - AFT_LAG < NCH:
                    emit_aft(t - AFT_LAG)
    nc.compile()
    return nc


def _prep(inputs):
    key = "prog"
    if key not in _CACHE:
        meta = _build_host(
            inputs["me_fir"], inputs["lin_fir"], inputs["nlin_fir_before"],
            inputs["nlin_fir_after"], inputs["lpf_lin_b"], inputs["lpf_lin_a"],
            inputs["lpf_nlin_b"], inputs["lpf_nlin_a"],
            np.asarray(inputs["lin_gain"], np.float64),
            np.asarray(inputs["nlin_a"], np.float64),
            np.asarray(inputs["nlin_b"], np.float64),
        )
        _CACHE[key] = (meta, _build_program(meta))
    return _CACHE[key]


def _in_maps(meta, x):
    xf = _fold_x(np.asarray(x, np.float32)).reshape(P, -1)
    XSEC = PADX + R
    head = np.ascontiguousarray(
        np.concatenate([meta["wme"], xf[:, :3 * XSEC]], axis=1))
    xfr = np.ascontiguousarray(xf[:, 3 * XSEC:])
    return [
        {"head": head, "xfr": xfr,
         "blobA": meta["blobA"][c].astype(BF16),
         "blobB": meta["blobB"][c].astype(BF16),
         "blob8": meta["blob8"][c],
         "scal": np.ascontiguousarray(np.broadcast_to(meta["scal"][c], (P, N_SLOTS * 4)))}
        for c in range(N_CORES)
    ]


def _decode(meta, youts):
    PADS = meta["PADS"]
    SEC = PADS + R
    WW = SEC + R
    W3 = 2 * SEC + R
    slot_ch = meta["slot_ch"]
    out = np.zeros((B, F, T), np.float32)
    for c in range(N_CORES):
        yo = np.asarray(youts[c], dtype=np.float32)
        for j, (s, h) in enumerate(CHUNKS):
            if s == 6 and c >= 2:
                continue
            f = slot_ch[c, s]
            inv = 1.0 / meta["gam"][c, s]
            for bi in range(NSEC_H[h]):
                b = QH[h] + bi
                col = (j % 3) * W3 + bi * SEC
                out[b, f, :] = yo[j // 3, :, col:col + R].T.reshape(R * P)[:T] * inv
    return out


def kernel(**inputs):
    meta, nc = _prep(inputs)
    from concourse.bass_utils import run_bass_kernel_spmd

    res = run_bass_kernel_spmd(nc, _in_maps(meta, inputs["x"]),
                               core_ids=list(range(N_CORES)),
                               trace=bool(inputs.get("_trace", False)))
    out = _decode(meta, [res.results[c]["yout"] for c in range(N_CORES)])
    if inputs.get("_return_res", False):
        return out, res
    return out


# revision 103
# speedup vs baseline: 1.0384x; 1.0384x over previous
"""DRNL filterbank Trainium2 kernel, v5.

Banded-Toeplitz formulation (ME/LIN/AFT bf16 band matmuls, BEF fp8
DoubleRow pairs, broken-stick pointwise on vector+scalar).

v5: 3-section chunks. A matmul's psum output must fit one 2KB bank
(512 f32) and the 3-section width 2*SEC+R = 491 fits exactly, so each
LIN/BEF/AFT band runs once per 3 batches instead of once per 2
(28 -> 21 chunks, 264 -> 198 matmuls/core) while keeping the original
1-bank psum rotation (o x4, v x3, me). 4-section (658) fails the ISA
check. The ~140ns fixed per-matmul cost makes fewer-but-wider strictly
better. ME also runs in 3-batch groups aligned with the chunk groups.
Startup: each DMA costs ~1.3us of DMA-engine pipeline latency, so the
whole first-matmul critical path (wme + ME group 0's xf slice) ships
as ONE "head" DMA on the SP queue; the Act queue streams blob8/blobB
in parallel on its own DMA engine. Do NOT shrink the work pool below
bufs=4 (bufs=2 miscomputes), do NOT truncate the ME FIR (1e-3 trunc
-> 1.7e-2 rel err; per-channel LIN gains amplify it), and do NOT put
sqrt-mode on the light tail chunks (serial 3-act latency stalls AFT).

v4 vs v3:
  - DMA consolidation: weight blobs packed in first-use order (bf16
    LIN/AFT blobs + fp8 BEF blob, blobB split in two), one contiguous
    xf DMA (k-major host fold), host-replicated scal, quad output DMAs.
    54 -> 14 DMA_DIRECT2D issues across both HWDGE queues (SP carries
    wme/xf/scal + outputs, Activation carries the weight blobs so the
    first-matmul path skips the ACT_TABLE_LOAD stall).
  - Truncation tolerances raised to 4e-3 / 8e-2 (292 -> 264 matmuls):
    the output error is floored by the final bf16 output-copy
    quantization at the response peak (~4e-3 rel), so shorter IR tails
    are free until truncation error approaches that floor.
  - fp8 DoubleRow for AFT was tried and reverted: pair matmuls measure
    ~450ns vs ~280ns bf16 singles (DoubleRow disables FWL and the dual
    rhs stream is fetch-bound), so the win never materializes, and the
    required one-column-shifted fp8 w copy stalls the PE. The PE also
    rejects small pair-dim rhs strides at runtime (dup_off=-1 aborts),
    so a second shifted signal copy is mandatory for any pair rhs.
Sharding: channels across 8 cores, slot-structured SPMD.
"""
import numpy as np
import ml_dtypes

P = 128
B, T, F = 8, 20000, 50
R = (T + P - 1) // P
N_CORES = 8
N_SLOTS = 7
TRUNC_LIN = 4e-3
TRUNC_NL = 8e-2
IR_LEN = 4096
BME = 5
PADX = 4

_CACHE = {}

BF16 = ml_dtypes.bfloat16
FP8 = ml_dtypes.float8_e4m3fn

# 3-section chunks: a matmul's psum output must stay within one 2KB bank
# (512 f32), and 2*SEC+WW = 491 fits exactly -- so each chunk covers 3
# batches (the last group covers the 2 remaining). 21 chunks total.
CHUNK_SLOTS = [4, 0, 6, 1, 5, 2, 3]
_GROUP_ORDERS = {0: [4, 0, 6, 1, 5, 2, 3], 1: [0, 6, 1, 5, 2, 4, 3],
                 2: [0, 6, 1, 5, 2, 3, 4]}
CHUNKS = [(s, h) for h in (0, 1, 2) for s in _GROUP_ORDERS[h]]
NCH = len(CHUNKS)
NSEC_H = [3, 3, 2]   # batches (sections groups) per h
QH = [0, 3, 6]       # first batch of each group
# pointwise engine schedule (tuned against the trace):
C_MODE = ["int" if (j * 14) % 21 < 14 else "sqrt" for j in range(NCH)]
MT_ENG = ["vector"] * NCH           # STT min(a*v, c)   (PSUM read -> vector only)
WT_ENG = ["vector"] * NCH           # STT w = max(-c, m) (Pool rejects STT entirely)
OC_ENG = ["scalar"] * NCH           # psum -> bf16 output copy
XC_ENG = ["gpsimd", "gpsimd"]       # xme8 main/shifted cast engines
BEF_LAG = 1   # BEF_j issues one chunk-slot after LIN_j
AFT_LAG = 3   # AFT_j issues in iteration j+AFT_LAG
BLOBA_SLOTS = CHUNK_SLOTS[:2]
# blob B packed in first-use order (LIN at slot's first chunk, AFT 3 chunks
# later), split into two DMAs after the 5th block so early consumers don't
# wait on the full transfer
BLOBB_ORDER = [("lin", 6), ("lin", 1), ("aft", 4), ("lin", 5), ("aft", 0),
               ("lin", 2), ("aft", 6), ("lin", 3), ("aft", 1), ("aft", 5),
               ("aft", 2), ("aft", 3)]
BLOBB_SPLIT = 5


def _lfilter_vec(x, b, a):
    b0, b1, b2 = b[:, 0], b[:, 1], b[:, 2]
    a1, a2 = a[:, 1], a[:, 2]
    y = np.zeros_like(x)
    z1 = np.zeros(x.shape[0])
    z2 = np.zeros(x.shape[0])
    for t in range(x.shape[-1]):
        xt = x[:, t]
        yt = b0 * xt + z1
        z1 = b1 * xt - a1 * yt + z2
        z2 = b2 * xt - a2 * yt
        y[:, t] = yt
    return y


def _cascade_ir(b, a, n, times):
    h = np.zeros((b.shape[0], n))
    h[:, 0] = 1.0
    for _ in range(times):
        h = _lfilter_vec(h, b, a)
    return h


def _trunc(h, tol):
    m = np.abs(h).max()
    idx = np.nonzero(np.abs(h) > tol * m)[0]
    return h[: int(idx[-1]) + 1] if len(idx) else h[:1]


def _nb(h):
    return (len(h) + P - 2) // P + 1


def _toeplitz_band(h, d):
    k = np.arange(P)[:, None]
    i = np.arange(P)[None, :]
    idx = P * d + i - k
    ok = (idx >= 0) & (idx < len(h))
    return np.where(ok, np.asarray(h, np.float64)[np.clip(idx, 0, len(h) - 1)], 0.0)


def _bands_cat(h, nb, dtype):
    W = np.concatenate([_toeplitz_band(h, d) for d in range(nb)], axis=1)
    return np.ascontiguousarray(W).astype(dtype)


def _pairs_cat(h, nbp, dtype):
    mats = []
    for p in range(nbp):
        mats.append(_toeplitz_band(h, 2 * p))
        mats.append(_toeplitz_band(h, 2 * p + 1))
    W = np.concatenate(mats, axis=1)
    return np.ascontiguousarray(W).astype(dtype)


def _tune_k(b):
    """Best int32 bias K: bitcast((bits(x)>>2)+K) ~= b*x**0.25 over x range."""
    x = np.float32(np.logspace(-6, 1.5, 4000))
    t1 = (x.view(np.int32) & 0x7FFFFFFF) >> 2
    ref = np.float64(b) * np.float64(x) ** 0.25
    b0 = 127 << 23
    base = np.int64(np.float32(b).view(np.int32)) - b0 // 4
    best = (1e9, 0)
    for sig in np.linspace(-0.02, 0.10, 121):
        K = np.int64(base - int(sig * (1 << 23)))
        c = (t1.astype(np.int64) + K).astype(np.int32).view(np.float32)
        e = np.abs(c - ref) / ref
        best = min(best, (float(e.max()), int(K)))
    return best[1]


def _build_host(me_fir, lin_fir, nlin_fir_before, nlin_fir_after,
                lpf_lin_b, lpf_lin_a, lpf_nlin_b, lpf_nlin_a,
                lin_gain, nlin_a, nlin_b):
    ir4 = _cascade_ir(lpf_lin_b.astype(np.float64), lpf_lin_a.astype(np.float64), IR_LEN, 4)
    ir3 = _cascade_ir(lpf_nlin_b.astype(np.float64), lpf_nlin_a.astype(np.float64), IR_LEN, 3)

    scale = 10.0 ** ((93.98 - 100.0) / 20.0)
    ME = np.asarray(me_fir, np.float64) * scale
    bme = _nb(ME)

    LIN, BEF, AFT = [], [], []
    for f in range(F):
        LIN.append(_trunc(lin_gain[f] * np.convolve(np.asarray(lin_fir[f], np.float64), ir4[f]), TRUNC_LIN))
        BEF.append(_trunc(np.asarray(nlin_fir_before[f], np.float64), TRUNC_NL))
        AFT.append(_trunc(np.convolve(np.asarray(nlin_fir_after[f], np.float64), ir3[f]), TRUNC_NL))

    nbp = lambda h: (_nb(h) + 1) // 2
    cost = [_nb(LIN[f]) + nbp(BEF[f]) + _nb(AFT[f]) for f in range(F)]
    order = np.argsort(-np.asarray(cost), kind="stable")

    slot_ch = np.zeros((N_CORES, N_SLOTS), np.int64)
    for s in range(6):
        for c in range(N_CORES):
            slot_ch[c, s] = order[8 * s + c]
    for c in range(N_CORES):
        slot_ch[c, 6] = order[48 + (c % 2)]

    BL = [max(_nb(LIN[slot_ch[c, s]]) for c in range(N_CORES)) for s in range(N_SLOTS)]
    BBp = [max(nbp(BEF[slot_ch[c, s]]) for c in range(N_CORES)) for s in range(N_SLOTS)]
    BA = [max(_nb(AFT[slot_ch[c, s]]) for c in range(N_CORES)) for s in range(N_SLOTS)]

    sh = max(max(BL) - 1, 2 * max(BBp) - 1, max(BA) - 1)
    PADS = (sh + 1 + 1) // 2 * 2

    gam = np.ones((N_CORES, N_SLOTS), np.float64)

    wme = _bands_cat(ME, bme, BF16)
    blobA, blobB, blob8 = [], [], []
    for c in range(N_CORES):
        a16 = [_bands_cat(gam[c, s] * LIN[slot_ch[c, s]], BL[s], BF16)
               for s in BLOBA_SLOTS]
        b16 = []
        for kind, s in BLOBB_ORDER:
            h = (gam[c, s] * LIN[slot_ch[c, s]] if kind == "lin"
                 else gam[c, s] * AFT[slot_ch[c, s]])
            b16.append(_bands_cat(h, BL[s] if kind == "lin" else BA[s], BF16))
        p8 = [_pairs_cat(BEF[slot_ch[c, s]], BBp[s], FP8) for s in CHUNK_SLOTS]
        blobA.append(np.concatenate(a16, axis=1))
        blobB.append(np.concatenate(b16, axis=1))
        blob8.append(np.concatenate(p8, axis=1))

    scal = np.zeros((N_CORES, N_SLOTS * 4), np.float32)
    for c in range(N_CORES):
        for s in range(N_SLOTS):
            f = slot_ch[c, s]
            scal[c, 4 * s + 0] = nlin_a[f]
            scal[c, 4 * s + 1] = float(nlin_b[f]) ** 4
            scal[c].view(np.int32)[4 * s + 2] = _tune_k(float(nlin_b[f]))

    return {
        "slot_ch": slot_ch, "BL": BL, "BBp": BBp, "BA": BA, "PADS": PADS,
        "gam": gam, "BME": bme,
        "wme": wme, "blobA": blobA, "blobB": blobB, "blob8": blob8,
        "scal": scal,
    }


def _fold_x(x):
    # k-major layout [P, B, XSEC]: the xf DMA is 128 contiguous rows
    xp = np.zeros((B, R * P), np.float32)
    xp[:, :T] = x
    xf = np.zeros((P, B, PADX + R), np.float32)
    xf[:, :, PADX:] = xp.reshape(B, R, P).transpose(2, 0, 1)
    return np.ascontiguousarray(xf).astype(BF16)


def _build_program(meta):
    import concourse.bacc as bacc
    import concourse.bass as bass
    from concourse import mybir
    from concourse.tile import TileContext

    BL, BBp, BA, PADS = meta["BL"], meta["BBp"], meta["BA"], meta["PADS"]
    BME = meta["BME"]
    SEC = PADS + R
    WW = SEC + R
    W3 = 2 * SEC + R     # 3-section chunk width (491 f32: fits one psum bank)
    WH = [W3, W3, WW]    # matmul/pointwise width per group h
    XSEC = PADX + R
    XW = XSEC + R
    f32, bf16, f8 = mybir.dt.float32, mybir.dt.bfloat16, mybir.dt.float8e4
    i32 = mybir.dt.int32
    AF = mybir.ActivationFunctionType
    ALU = mybir.AluOpType
    DR = mybir.MatmulPerfMode.DoubleRow

    olA, olB, ob8, oa = {}, {}, {}, {}
    off = 0
    for s in BLOBA_SLOTS:
        olA[s] = off
        off += BL[s]
    nA = off
    off = 0
    nB_split = 0
    for bi, (kind, s) in enumerate(BLOBB_ORDER):
        if kind == "lin":
            olB[s] = off
            off += BL[s]
        else:
            oa[s] = off
            off += BA[s]
        if bi + 1 == BLOBB_SPLIT:
            nB_split = off
    nB = off
    off = 0
    for s in CHUNK_SLOTS:
        ob8[s] = off
        off += BBp[s]
    n8 = off

    HEADW = BME * P + 3 * XSEC
    nc = bacc.Bacc("TRN2", target_bir_lowering=False, debug=False, num_devices=N_CORES)
    # head = wme bands + ME group 0's xf slice: the whole first-matmul
    # critical path arrives with a single DMA (each extra DMA adds ~1.3us
    # of DMA-engine pipeline latency)
    d_head = nc.dram_tensor("head", [P, HEADW], bf16, kind="ExternalInput").ap()
    d_xfr = nc.dram_tensor("xfr", [P, 5 * XSEC], bf16, kind="ExternalInput").ap()
    d_blobA = nc.dram_tensor("blobA", [P, nA * P], bf16, kind="ExternalInput").ap()
    d_blobB = nc.dram_tensor("blobB", [P, nB * P], bf16, kind="ExternalInput").ap()
    d_blob8 = nc.dram_tensor("blob8", [P, n8 * 2 * P], f8, kind="ExternalInput").ap()
    d_scal = nc.dram_tensor("scal", [P, N_SLOTS * 4], f32, kind="ExternalInput").ap()
    d_out = nc.dram_tensor("yout", [7, P, 3 * W3], bf16, kind="ExternalOutput").ap()

    def pair_rhs(tile, col_off, dup_off):
        base = tile[:, 0:1]
        return bass.AP(tensor=base.tensor, offset=base.offset + col_off,
                       ap=[[base.ap[0][0], P], [dup_off, 2], [1, WW]])

    def pair_lhs(tile, band_off, p):
        s = tile[:, (band_off + p) * 2 * P:(band_off + p + 1) * 2 * P]
        return bass.AP(tensor=s.tensor, offset=s.offset,
                       ap=[[s.ap[0][0], P], [P, 2], [1, P]])

    ENG = None  # set inside context

    with TileContext(nc) as tc:
        with (
            tc.tile_pool(name="singles", bufs=1) as singles,
            tc.tile_pool(name="work", bufs=4) as work,
            tc.tile_pool(name="ps", bufs=1, space="PSUM") as ps,
        ):
            ENG = {"vector": nc.vector, "scalar": nc.scalar, "gpsimd": nc.gpsimd}

            def ecopy(eng, out, in_):
                if eng == "scalar":
                    nc.scalar.activation(out, in_, AF.Copy)
                else:
                    ENG[eng].tensor_copy(out=out, in_=in_)
            # SP: head (first-matmul critical path), blobA (LIN chunk 0),
            # xf rest. Act: scal, then the remaining weight blobs — the two
            # queues transfer on separate DMA engines.
            head_t = singles.tile([P, HEADW], bf16)
            nc.sync.dma_start(out=head_t, in_=d_head)
            blobA_t = singles.tile([P, nA * P], bf16)
            nc.sync.dma_start(out=blobA_t, in_=d_blobA)
            xfr_t = singles.tile([P, 5 * XSEC], bf16)
            nc.sync.dma_start(out=xfr_t, in_=d_xfr)
            blob8_t = singles.tile([P, n8 * 2 * P], f8)
            nc.scalar.dma_start(out=blob8_t, in_=d_blob8)
            scal_t = singles.tile([P, N_SLOTS * 4], f32)
            nc.scalar.dma_start(out=scal_t, in_=d_scal)
            blobB_t = singles.tile([P, nB * P], bf16)
            nc.scalar.dma_start(out=blobB_t[:, :nB_split * P],
                                in_=d_blobB[:, :nB_split * P])
            nc.scalar.dma_start(out=blobB_t[:, nB_split * P:],
                                in_=d_blobB[:, nB_split * P:])

            def wl_slice(s, d):
                if s in olA:
                    return blobA_t[:, (olA[s] + d) * P:(olA[s] + d + 1) * P]
                return blobB_t[:, (olB[s] + d) * P:(olB[s] + d + 1) * P]

            def wa_slice(s, d):
                return blobB_t[:, (oa[s] + d) * P:(oa[s] + d + 1) * P]

            O8 = (B * SEC + 7) // 4 * 4
            xme16 = singles.tile([P, B * SEC], bf16)
            xme8 = singles.tile([P, O8 + B * SEC + 4], f8)
            for qq in range(B):
                nc.vector.memset(xme16[:, qq * SEC:qq * SEC + PADS], 0.0)
                nc.gpsimd.memset(xme8[:, qq * SEC:qq * SEC + PADS], 0.0)
                lo = O8 + qq * SEC + (1 if qq else 0)
                nc.gpsimd.memset(xme8[:, lo:O8 + qq * SEC + PADS + 1], 0.0)

            # ME in 3-batch groups aligned with the chunk groups: group h
            # produces exactly the xme sections chunk group h consumes
            XW3 = 2 * XSEC + R

            def emit_me(g):
                q = QH[g]
                n = NSEC_H[g]
                xw = (n - 1) * XSEC + R
                mp = ps.tile([P, XW3], f32, tag="me")
                for d in range(BME):
                    if g == 0:
                        ws = BME * P + q * XSEC + PADX - d
                        rhs = head_t[:, ws:ws + xw]
                    else:
                        ws = (q - 3) * XSEC + PADX - d
                        rhs = xfr_t[:, ws:ws + xw]
                    nc.tensor.matmul(mp[:, 0:xw], head_t[:, d * P:(d + 1) * P],
                                     rhs, start=(d == 0), stop=(d == BME - 1))
                for bi in range(n):
                    qq = q + bi
                    # spread the psum->xme16 copies across vector+scalar so
                    # the chunk's LIN (which needs all n sections) starts
                    # ~0.4us sooner than with a serial vector chain
                    ecopy("scalar" if bi == 1 else "vector",
                          xme16[:, qq * SEC + PADS:(qq + 1) * SEC],
                          mp[:, bi * XSEC:bi * XSEC + R])
                    src = xme16[:, qq * SEC + PADS:(qq + 1) * SEC]
                    ecopy(XC_ENG[0], xme8[:, qq * SEC + PADS:(qq + 1) * SEC], src)
                    ecopy(XC_ENG[1],
                          xme8[:, O8 + qq * SEC + PADS + 1:O8 + (qq + 1) * SEC + 1],
                          src)

            emit_me(0)

            o_psd, v_psd, w_td = {}, {}, {}
            w_bufs = []
            for i in range(AFT_LAG + 1):
                wbuf = singles.tile([P, 3 * SEC], bf16, tag=f"wbuf{i}")
                nc.vector.memset(wbuf[:, 0:PADS], 0.0)
                w_bufs.append(wbuf)
            oq_tiles = []
            for i in range(2):
                oq_t = singles.tile([P, 3 * W3], bf16, tag=f"oq{i}")
                oq_tiles.append(oq_t)

            def emit_lin(j):
                s, h = CHUNKS[j]
                q = QH[h]
                o_ps = ps.tile([P, W3], f32, tag=f"o{j % 4}")
                o_psd[j] = o_ps
                for d in range(BL[s]):
                    ws = q * SEC + PADS - d
                    nc.tensor.matmul(o_ps[:, 0:WH[h]], wl_slice(s, d),
                                     xme16[:, ws:ws + WH[h]],
                                     start=(d == 0), stop=False)

            def emit_bef_pointwise(j):
                s, h = CHUNKS[j]
                q = QH[h]
                wd = WH[h]
                a_ap = scal_t[:, 4 * s + 0:4 * s + 1]
                b4_ap = scal_t[:, 4 * s + 1:4 * s + 2]
                k_ap = scal_t.bitcast(i32)[:, 4 * s + 2:4 * s + 3]
                v_ps = ps.tile([P, W3], f32, tag=f"v{j % 3}")
                v_psd[j] = v_ps
                for p in range(BBp[s]):
                    base = xme8[:, 0:1]
                    rhs = bass.AP(tensor=base.tensor,
                                  offset=base.offset + q * SEC + PADS - 2 * p,
                                  ap=[[base.ap[0][0], P], [O8, 2], [1, wd]])
                    nc.tensor.matmul(v_ps[:, 0:wd], pair_lhs(blob8_t, ob8[s], p),
                                     rhs,
                                     start=(p == 0), stop=(p == BBp[s] - 1),
                                     perf_mode=DR)
                c_t = work.tile([P, W3], f32, tag="c")
                if C_MODE[j] == "sqrt":
                    u_t = work.tile([P, W3], f32, tag="u")
                    nc.scalar.activation(u_t[:, 0:wd], v_ps[:, 0:wd], AF.Abs)
                    nc.scalar.activation(c_t[:, 0:wd], u_t[:, 0:wd], AF.Sqrt,
                                         scale=b4_ap)
                    nc.scalar.sqrt(c_t[:, 0:wd], c_t[:, 0:wd])
                else:
                    t1 = work.tile([P, W3], i32, tag="u")
                    nc.vector.tensor_scalar(
                        out=t1[:, 0:wd], in0=v_ps[:, 0:wd].bitcast(i32),
                        scalar1=0x7FFFFFFF, op0=ALU.bitwise_and,
                        scalar2=2, op1=ALU.logical_shift_right)
                    nc.scalar.activation(c_t[:, 0:wd].bitcast(i32), t1[:, 0:wd],
                                         AF.Identity, bias=k_ap)
                m_t = work.tile([P, W3], f32, tag="m")
                ENG[MT_ENG[j]].scalar_tensor_tensor(
                    out=m_t[:, 0:wd], in0=v_ps[:, 0:wd], scalar=a_ap,
                    in1=c_t[:, 0:wd], op0=ALU.mult, op1=ALU.min,
                )
                w_t = w_bufs[j % len(w_bufs)]
                w_td[j] = w_t
                ENG[WT_ENG[j]].scalar_tensor_tensor(
                    out=w_t[:, PADS:PADS + wd], in0=c_t[:, 0:wd], scalar=-1.0,
                    in1=m_t[:, 0:wd], op0=ALU.mult, op1=ALU.max,
                )
                # re-zero the mid pads the stt overwrote (1 or 2 of them),
                # on the same queue as the stt
                pm = w_t[:, SEC:SEC + 1]
                ENG[WT_ENG[j]].memset(
                    bass.AP(tensor=pm.tensor, offset=pm.offset,
                            ap=[[pm.ap[0][0], P], [SEC, NSEC_H[h] - 1],
                                [1, PADS]]), 0.0)

            def emit_aft(j):
                s, h = CHUNKS[j]
                wd = WH[h]
                o_ps, w_t = o_psd.pop(j), w_td.pop(j)
                for d in range(BA[s]):
                    ws = PADS - d
                    nc.tensor.matmul(o_ps[:, 0:wd], wa_slice(s, d),
                                     w_t[:, ws:ws + wd],
                                     start=False, stop=(d == BA[s] - 1))
                oq = oq_tiles[(j // 3) % 2]
                out_t = oq[:, (j % 3) * W3:(j % 3) * W3 + wd]
                ecopy(OC_ENG[j], out_t, o_ps[:, 0:wd])
                if j % 3 == 2 or j == NCH - 1:
                    nc.sync.dma_start(out=d_out[j // 3], in_=oq)

            for t in range(NCH + AFT_LAG):
                if t == 2:
                    emit_me(1)
                if t == 8:
                    emit_me(2)
                if t < NCH:
                    emit_lin(t)
                if 0 <= t - BEF_LAG < NCH:
                    emit_bef_pointwise(t - BEF_LAG)
                if 0 <= t - AFT_LAG < NCH:
                    emit_aft(t - AFT_LAG)
    nc.compile()
    return nc


def _prep(inputs):
    key = "prog"
    if key not in _CACHE:
        meta = _build_host(
            inputs["me_fir"], inputs["lin_fir"], inputs["nlin_fir_before"],
            inputs["nlin_fir_after"], inputs["lpf_lin_b"], inputs["lpf_lin_a"],
            inputs["lpf_nlin_b"], inputs["lpf_nlin_a"],
            np.asarray(inputs["lin_gain"], np.float64),
            np.asarray(inputs["nlin_a"], np.float64),
            np.asarray(inputs["nlin_b"], np.float64),
        )
        _CACHE[key] = (meta, _build_program(meta))
    return _CACHE[key]


def _in_maps(meta, x):
    xf = _fold_x(np.asarray(x, np.float32)).reshape(P, -1)
    XSEC = PADX + R
    head = np.ascontiguousarray(
        np.concatenate([meta["wme"], xf[:, :3 * XSEC]], axis=1))
    xfr = np.ascontiguousarray(xf[:, 3 * XSEC:])
    return [
        {"head": head, "xfr": xfr,
         "blobA": meta["blobA"][c].astype(BF16),
         "blobB": meta["blobB"][c].astype(BF16),
         "blob8": meta["blob8"][c],
         "scal": np.ascontiguousarray(np.broadcast_to(meta["scal"][c], (P, N_SLOTS * 4)))}
        for c in range(N_CORES)
    ]


def _decode(meta, youts):
    PADS = meta["PADS"]
    SEC = PADS + R
    WW = SEC + R
    W3 = 2 * SEC + R
    slot_ch = meta["slot_ch"]
    out = np.zeros((B, F, T), np.float32)
    for c in range(N_CORES):
        yo = np.asarray(youts[c], dtype=np.float32)
        for j, (s, h) in enumerate(CHUNKS):
            if s == 6 and c >= 2:
                continue
            f = slot_ch[c, s]
            inv = 1.0 / meta["gam"][c, s]
            for bi in range(NSEC_H[h]):
                b = QH[h] + bi
                col = (j % 3) * W3 + bi * SEC
                out[b, f, :] = yo[j // 3, :, col:col + R].T.reshape(R * P)[:T] * inv
    return out


def kernel(**inputs):
    meta, nc = _prep(inputs)
    from concourse.bass_utils import run_bass_kernel_spmd

    res = run_bass_kernel_spmd(nc, _in_maps(meta, inputs["x"]),
                               core_ids=list(range(N_CORES)),
                               trace=bool(inputs.get("_trace", False)))
    out = _decode(meta, [res.results[c]["yout"] for c in range(N_CORES)])
    if inputs.get("_return_res", False):
        return out, res
    return out


# revision 105
# speedup vs baseline: 1.0457x; 1.0070x over previous
"""DRNL filterbank Trainium2 kernel, v5.

Banded-Toeplitz formulation (ME/LIN/AFT bf16 band matmuls, BEF fp8
DoubleRow pairs, broken-stick pointwise on vector+scalar).

v5: 3-section chunks. A matmul's psum output must fit one 2KB bank
(512 f32) and the 3-section width 2*SEC+R = 491 fits exactly, so each
LIN/BEF/AFT band runs once per 3 batches instead of once per 2
(28 -> 21 chunks, 264 -> 198 matmuls/core) while keeping the original
1-bank psum rotation (o x4, v x3, me). 4-section (658) fails the ISA
check. The ~140ns fixed per-matmul cost makes fewer-but-wider strictly
better. ME also runs in 3-batch groups aligned with the chunk groups.
Startup: each DMA costs ~1.3us of DMA-engine pipeline latency, so the
whole first-matmul critical path (wme + ME group 0's xf slice) ships
as ONE "head" DMA on the SP queue; the Act queue streams blob8/blobB
in parallel on its own DMA engine. Do NOT shrink the work pool below
bufs=4 (bufs=2 miscomputes), do NOT truncate the ME FIR (1e-3 trunc
-> 1.7e-2 rel err; per-channel LIN gains amplify it), and do NOT put
sqrt-mode on the light tail chunks (serial 3-act latency stalls AFT).

v4 vs v3:
  - DMA consolidation: weight blobs packed in first-use order (bf16
    LIN/AFT blobs + fp8 BEF blob, blobB split in two), one contiguous
    xf DMA (k-major host fold), host-replicated scal, quad output DMAs.
    54 -> 14 DMA_DIRECT2D issues across both HWDGE queues (SP carries
    wme/xf/scal + outputs, Activation carries the weight blobs so the
    first-matmul path skips the ACT_TABLE_LOAD stall).
  - Truncation tolerances raised to 4e-3 / 8e-2 (292 -> 264 matmuls):
    the output error is floored by the final bf16 output-copy
    quantization at the response peak (~4e-3 rel), so shorter IR tails
    are free until truncation error approaches that floor.
  - fp8 DoubleRow for AFT was tried and reverted: pair matmuls measure
    ~450ns vs ~280ns bf16 singles (DoubleRow disables FWL and the dual
    rhs stream is fetch-bound), so the win never materializes, and the
    required one-column-shifted fp8 w copy stalls the PE. The PE also
    rejects small pair-dim rhs strides at runtime (dup_off=-1 aborts),
    so a second shifted signal copy is mandatory for any pair rhs.
Sharding: channels across 8 cores, slot-structured SPMD.
"""
import numpy as np
import ml_dtypes

P = 128
B, T, F = 8, 20000, 50
R = (T + P - 1) // P
N_CORES = 8
N_SLOTS = 7
TRUNC_LIN = 4e-3
TRUNC_NL = 8e-2
IR_LEN = 4096
BME = 5
PADX = 4

_CACHE = {}

BF16 = ml_dtypes.bfloat16
FP8 = ml_dtypes.float8_e4m3fn

# 3-section chunks: a matmul's psum output must stay within one 2KB bank
# (512 f32), and 2*SEC+WW = 491 fits exactly -- so each chunk covers 3
# batches (the last group covers the 2 remaining). 21 chunks total.
CHUNK_SLOTS = [4, 0, 6, 1, 5, 2, 3]
_GROUP_ORDERS = {0: [4, 0, 6, 1, 5, 2, 3], 1: [0, 6, 1, 5, 2, 4, 3],
                 2: [0, 6, 1, 5, 2, 3, 4]}
CHUNKS = [(s, h) for h in (0, 1, 2) for s in _GROUP_ORDERS[h]]
NCH = len(CHUNKS)
NSEC_H = [3, 3, 2]   # batches (sections groups) per h
QH = [0, 3, 6]       # first batch of each group
# pointwise engine schedule (tuned against the trace):
# sqrt chunks shifted off the observed AFT-stall positions; the final
# chunk stays int so its w latency doesn't lengthen the drain
C_MODE = ["sqrt" if (j % 3 == 2 and j < 20) else "int" for j in range(NCH)]
MT_ENG = ["vector"] * NCH           # STT min(a*v, c)   (PSUM read -> vector only)
WT_ENG = ["vector"] * NCH           # STT w = max(-c, m) (Pool rejects STT entirely)
OC_ENG = ["scalar"] * NCH           # psum -> bf16 output copy
XC_ENG = ["gpsimd", "gpsimd"]       # xme8 main/shifted cast engines
BEF_LAG = 1   # BEF_j issues one chunk-slot after LIN_j
AFT_LAG = 3   # AFT_j issues in iteration j+AFT_LAG
BLOBA_SLOTS = CHUNK_SLOTS[:2]
# blob B packed in first-use order (LIN at slot's first chunk, AFT 3 chunks
# later), split into two DMAs after the 5th block so early consumers don't
# wait on the full transfer
BLOBB_ORDER = [("lin", 6), ("lin", 1), ("aft", 4), ("lin", 5), ("aft", 0),
               ("lin", 2), ("aft", 6), ("lin", 3), ("aft", 1), ("aft", 5),
               ("aft", 2), ("aft", 3)]
BLOBB_SPLIT = 5


def _lfilter_vec(x, b, a):
    b0, b1, b2 = b[:, 0], b[:, 1], b[:, 2]
    a1, a2 = a[:, 1], a[:, 2]
    y = np.zeros_like(x)
    z1 = np.zeros(x.shape[0])
    z2 = np.zeros(x.shape[0])
    for t in range(x.shape[-1]):
        xt = x[:, t]
        yt = b0 * xt + z1
        z1 = b1 * xt - a1 * yt + z2
        z2 = b2 * xt - a2 * yt
        y[:, t] = yt
    return y


def _cascade_ir(b, a, n, times):
    h = np.zeros((b.shape[0], n))
    h[:, 0] = 1.0
    for _ in range(times):
        h = _lfilter_vec(h, b, a)
    return h


def _trunc(h, tol):
    m = np.abs(h).max()
    idx = np.nonzero(np.abs(h) > tol * m)[0]
    return h[: int(idx[-1]) + 1] if len(idx) else h[:1]


def _nb(h):
    return (len(h) + P - 2) // P + 1


def _toeplitz_band(h, d):
    k = np.arange(P)[:, None]
    i = np.arange(P)[None, :]
    idx = P * d + i - k
    ok = (idx >= 0) & (idx < len(h))
    return np.where(ok, np.asarray(h, np.float64)[np.clip(idx, 0, len(h) - 1)], 0.0)


def _bands_cat(h, nb, dtype):
    W = np.concatenate([_toeplitz_band(h, d) for d in range(nb)], axis=1)
    return np.ascontiguousarray(W).astype(dtype)


def _pairs_cat(h, nbp, dtype):
    mats = []
    for p in range(nbp):
        mats.append(_toeplitz_band(h, 2 * p))
        mats.append(_toeplitz_band(h, 2 * p + 1))
    W = np.concatenate(mats, axis=1)
    return np.ascontiguousarray(W).astype(dtype)


def _tune_k(b):
    """Best int32 bias K: bitcast((bits(x)>>2)+K) ~= b*x**0.25 over x range."""
    x = np.float32(np.logspace(-6, 1.5, 4000))
    t1 = (x.view(np.int32) & 0x7FFFFFFF) >> 2
    ref = np.float64(b) * np.float64(x) ** 0.25
    b0 = 127 << 23
    base = np.int64(np.float32(b).view(np.int32)) - b0 // 4
    best = (1e9, 0)
    for sig in np.linspace(-0.02, 0.10, 121):
        K = np.int64(base - int(sig * (1 << 23)))
        c = (t1.astype(np.int64) + K).astype(np.int32).view(np.float32)
        e = np.abs(c - ref) / ref
        best = min(best, (float(e.max()), int(K)))
    return best[1]


def _build_host(me_fir, lin_fir, nlin_fir_before, nlin_fir_after,
                lpf_lin_b, lpf_lin_a, lpf_nlin_b, lpf_nlin_a,
                lin_gain, nlin_a, nlin_b):
    ir4 = _cascade_ir(lpf_lin_b.astype(np.float64), lpf_lin_a.astype(np.float64), IR_LEN, 4)
    ir3 = _cascade_ir(lpf_nlin_b.astype(np.float64), lpf_nlin_a.astype(np.float64), IR_LEN, 3)

    scale = 10.0 ** ((93.98 - 100.0) / 20.0)
    ME = np.asarray(me_fir, np.float64) * scale
    bme = _nb(ME)

    LIN, BEF, AFT = [], [], []
    for f in range(F):
        LIN.append(_trunc(lin_gain[f] * np.convolve(np.asarray(lin_fir[f], np.float64), ir4[f]), TRUNC_LIN))
        BEF.append(_trunc(np.asarray(nlin_fir_before[f], np.float64), TRUNC_NL))
        AFT.append(_trunc(np.convolve(np.asarray(nlin_fir_after[f], np.float64), ir3[f]), TRUNC_NL))

    nbp = lambda h: (_nb(h) + 1) // 2
    cost = [_nb(LIN[f]) + nbp(BEF[f]) + _nb(AFT[f]) for f in range(F)]
    order = np.argsort(-np.asarray(cost), kind="stable")

    slot_ch = np.zeros((N_CORES, N_SLOTS), np.int64)
    for s in range(6):
        for c in range(N_CORES):
            slot_ch[c, s] = order[8 * s + c]
    for c in range(N_CORES):
        slot_ch[c, 6] = order[48 + (c % 2)]

    BL = [max(_nb(LIN[slot_ch[c, s]]) for c in range(N_CORES)) for s in range(N_SLOTS)]
    BBp = [max(nbp(BEF[slot_ch[c, s]]) for c in range(N_CORES)) for s in range(N_SLOTS)]
    BA = [max(_nb(AFT[slot_ch[c, s]]) for c in range(N_CORES)) for s in range(N_SLOTS)]

    sh = max(max(BL) - 1, 2 * max(BBp) - 1, max(BA) - 1)
    PADS = (sh + 1 + 1) // 2 * 2

    gam = np.ones((N_CORES, N_SLOTS), np.float64)

    wme = _bands_cat(ME, bme, BF16)
    blobA, blobB, blob8 = [], [], []
    for c in range(N_CORES):
        a16 = [_bands_cat(gam[c, s] * LIN[slot_ch[c, s]], BL[s], BF16)
               for s in BLOBA_SLOTS]
        b16 = []
        for kind, s in BLOBB_ORDER:
            h = (gam[c, s] * LIN[slot_ch[c, s]] if kind == "lin"
                 else gam[c, s] * AFT[slot_ch[c, s]])
            b16.append(_bands_cat(h, BL[s] if kind == "lin" else BA[s], BF16))
        p8 = [_pairs_cat(BEF[slot_ch[c, s]], BBp[s], FP8) for s in CHUNK_SLOTS]
        blobA.append(np.concatenate(a16, axis=1))
        blobB.append(np.concatenate(b16, axis=1))
        blob8.append(np.concatenate(p8, axis=1))

    scal = np.zeros((N_CORES, N_SLOTS * 4), np.float32)
    for c in range(N_CORES):
        for s in range(N_SLOTS):
            f = slot_ch[c, s]
            scal[c, 4 * s + 0] = nlin_a[f]
            scal[c, 4 * s + 1] = float(nlin_b[f]) ** 4
            scal[c].view(np.int32)[4 * s + 2] = _tune_k(float(nlin_b[f]))

    return {
        "slot_ch": slot_ch, "BL": BL, "BBp": BBp, "BA": BA, "PADS": PADS,
        "gam": gam, "BME": bme,
        "wme": wme, "blobA": blobA, "blobB": blobB, "blob8": blob8,
        "scal": scal,
    }


def _fold_x(x):
    # k-major layout [P, B, XSEC]: the xf DMA is 128 contiguous rows
    xp = np.zeros((B, R * P), np.float32)
    xp[:, :T] = x
    xf = np.zeros((P, B, PADX + R), np.float32)
    xf[:, :, PADX:] = xp.reshape(B, R, P).transpose(2, 0, 1)
    return np.ascontiguousarray(xf).astype(BF16)


def _build_program(meta):
    import concourse.bacc as bacc
    import concourse.bass as bass
    from concourse import mybir
    from concourse.tile import TileContext

    BL, BBp, BA, PADS = meta["BL"], meta["BBp"], meta["BA"], meta["PADS"]
    BME = meta["BME"]
    SEC = PADS + R
    WW = SEC + R
    W3 = 2 * SEC + R     # 3-section chunk width (491 f32: fits one psum bank)
    WH = [W3, W3, WW]    # matmul/pointwise width per group h
    XSEC = PADX + R
    XW = XSEC + R
    f32, bf16, f8 = mybir.dt.float32, mybir.dt.bfloat16, mybir.dt.float8e4
    i32 = mybir.dt.int32
    AF = mybir.ActivationFunctionType
    ALU = mybir.AluOpType
    DR = mybir.MatmulPerfMode.DoubleRow

    olA, olB, ob8, oa = {}, {}, {}, {}
    off = 0
    for s in BLOBA_SLOTS:
        olA[s] = off
        off += BL[s]
    nA = off
    off = 0
    nB_split = 0
    for bi, (kind, s) in enumerate(BLOBB_ORDER):
        if kind == "lin":
            olB[s] = off
            off += BL[s]
        else:
            oa[s] = off
            off += BA[s]
        if bi + 1 == BLOBB_SPLIT:
            nB_split = off
    nB = off
    off = 0
    for s in CHUNK_SLOTS:
        ob8[s] = off
        off += BBp[s]
    n8 = off

    HEADW = BME * P + 3 * XSEC
    nc = bacc.Bacc("TRN2", target_bir_lowering=False, debug=False, num_devices=N_CORES)
    # head = wme bands + ME group 0's xf slice: the whole first-matmul
    # critical path arrives with a single DMA (each extra DMA adds ~1.3us
    # of DMA-engine pipeline latency)
    d_head = nc.dram_tensor("head", [P, HEADW], bf16, kind="ExternalInput").ap()
    d_xfr = nc.dram_tensor("xfr", [P, 5 * XSEC], bf16, kind="ExternalInput").ap()
    d_blobA = nc.dram_tensor("blobA", [P, nA * P], bf16, kind="ExternalInput").ap()
    d_blobB = nc.dram_tensor("blobB", [P, nB * P], bf16, kind="ExternalInput").ap()
    d_blob8 = nc.dram_tensor("blob8", [P, n8 * 2 * P], f8, kind="ExternalInput").ap()
    d_scal = nc.dram_tensor("scal", [P, N_SLOTS * 4], f32, kind="ExternalInput").ap()
    d_out = nc.dram_tensor("yout", [7, P, 3 * W3], bf16, kind="ExternalOutput").ap()

    def pair_rhs(tile, col_off, dup_off):
        base = tile[:, 0:1]
        return bass.AP(tensor=base.tensor, offset=base.offset + col_off,
                       ap=[[base.ap[0][0], P], [dup_off, 2], [1, WW]])

    def pair_lhs(tile, band_off, p):
        s = tile[:, (band_off + p) * 2 * P:(band_off + p + 1) * 2 * P]
        return bass.AP(tensor=s.tensor, offset=s.offset,
                       ap=[[s.ap[0][0], P], [P, 2], [1, P]])

    ENG = None  # set inside context

    with TileContext(nc) as tc:
        with (
            tc.tile_pool(name="singles", bufs=1) as singles,
            tc.tile_pool(name="work", bufs=4) as work,
            tc.tile_pool(name="ps", bufs=1, space="PSUM") as ps,
        ):
            ENG = {"vector": nc.vector, "scalar": nc.scalar, "gpsimd": nc.gpsimd}

            def ecopy(eng, out, in_):
                if eng == "scalar":
                    nc.scalar.activation(out, in_, AF.Copy)
                else:
                    ENG[eng].tensor_copy(out=out, in_=in_)
            # SP: head (first-matmul critical path), blobA (LIN chunk 0),
            # xf rest. Act: scal, then the remaining weight blobs — the two
            # queues transfer on separate DMA engines.
            head_t = singles.tile([P, HEADW], bf16)
            nc.sync.dma_start(out=head_t, in_=d_head)
            blobA_t = singles.tile([P, nA * P], bf16)
            nc.sync.dma_start(out=blobA_t, in_=d_blobA)
            xfr_t = singles.tile([P, 5 * XSEC], bf16)
            nc.sync.dma_start(out=xfr_t, in_=d_xfr)
            blob8_t = singles.tile([P, n8 * 2 * P], f8)
            nc.scalar.dma_start(out=blob8_t, in_=d_blob8)
            scal_t = singles.tile([P, N_SLOTS * 4], f32)
            nc.scalar.dma_start(out=scal_t, in_=d_scal)
            blobB_t = singles.tile([P, nB * P], bf16)
            nc.scalar.dma_start(out=blobB_t[:, :nB_split * P],
                                in_=d_blobB[:, :nB_split * P])
            nc.scalar.dma_start(out=blobB_t[:, nB_split * P:],
                                in_=d_blobB[:, nB_split * P:])

            def wl_slice(s, d):
                if s in olA:
                    return blobA_t[:, (olA[s] + d) * P:(olA[s] + d + 1) * P]
                return blobB_t[:, (olB[s] + d) * P:(olB[s] + d + 1) * P]

            def wa_slice(s, d):
                return blobB_t[:, (oa[s] + d) * P:(oa[s] + d + 1) * P]

            O8 = (B * SEC + 7) // 4 * 4
            xme16 = singles.tile([P, B * SEC], bf16)
            xme8 = singles.tile([P, O8 + B * SEC + 4], f8)
            for qq in range(B):
                nc.vector.memset(xme16[:, qq * SEC:qq * SEC + PADS], 0.0)
                nc.gpsimd.memset(xme8[:, qq * SEC:qq * SEC + PADS], 0.0)
                lo = O8 + qq * SEC + (1 if qq else 0)
                nc.gpsimd.memset(xme8[:, lo:O8 + qq * SEC + PADS + 1], 0.0)

            # ME in 3-batch groups aligned with the chunk groups: group h
            # produces exactly the xme sections chunk group h consumes
            XW3 = 2 * XSEC + R

            def emit_me(g):
                q = QH[g]
                n = NSEC_H[g]
                xw = (n - 1) * XSEC + R
                mp = ps.tile([P, XW3], f32, tag="me")
                for d in range(BME):
                    if g == 0:
                        ws = BME * P + q * XSEC + PADX - d
                        rhs = head_t[:, ws:ws + xw]
                    else:
                        ws = (q - 3) * XSEC + PADX - d
                        rhs = xfr_t[:, ws:ws + xw]
                    nc.tensor.matmul(mp[:, 0:xw], head_t[:, d * P:(d + 1) * P],
                                     rhs, start=(d == 0), stop=(d == BME - 1))
                for bi in range(n):
                    qq = q + bi
                    # spread the psum->xme16 copies across vector+scalar so
                    # the chunk's LIN (which needs all n sections) starts
                    # ~0.4us sooner than with a serial vector chain
                    ecopy("scalar" if bi == 1 else "vector",
                          xme16[:, qq * SEC + PADS:(qq + 1) * SEC],
                          mp[:, bi * XSEC:bi * XSEC + R])
                    src = xme16[:, qq * SEC + PADS:(qq + 1) * SEC]
                    ecopy(XC_ENG[0], xme8[:, qq * SEC + PADS:(qq + 1) * SEC], src)
                    ecopy(XC_ENG[1],
                          xme8[:, O8 + qq * SEC + PADS + 1:O8 + (qq + 1) * SEC + 1],
                          src)

            emit_me(0)

            o_psd, v_psd, w_td = {}, {}, {}
            w_bufs = []
            for i in range(AFT_LAG + 1):
                wbuf = singles.tile([P, 3 * SEC], bf16, tag=f"wbuf{i}")
                nc.vector.memset(wbuf[:, 0:PADS], 0.0)
                w_bufs.append(wbuf)
            oq_tiles = []
            for i in range(2):
                oq_t = singles.tile([P, 3 * W3], bf16, tag=f"oq{i}")
                oq_tiles.append(oq_t)

            def emit_lin(j):
                s, h = CHUNKS[j]
                q = QH[h]
                o_ps = ps.tile([P, W3], f32, tag=f"o{j % 4}")
                o_psd[j] = o_ps
                for d in range(BL[s]):
                    ws = q * SEC + PADS - d
                    nc.tensor.matmul(o_ps[:, 0:WH[h]], wl_slice(s, d),
                                     xme16[:, ws:ws + WH[h]],
                                     start=(d == 0), stop=False)

            def emit_bef_pointwise(j):
                s, h = CHUNKS[j]
                q = QH[h]
                wd = WH[h]
                a_ap = scal_t[:, 4 * s + 0:4 * s + 1]
                b4_ap = scal_t[:, 4 * s + 1:4 * s + 2]
                k_ap = scal_t.bitcast(i32)[:, 4 * s + 2:4 * s + 3]
                v_ps = ps.tile([P, W3], f32, tag=f"v{j % 3}")
                v_psd[j] = v_ps
                for p in range(BBp[s]):
                    base = xme8[:, 0:1]
                    rhs = bass.AP(tensor=base.tensor,
                                  offset=base.offset + q * SEC + PADS - 2 * p,
                                  ap=[[base.ap[0][0], P], [O8, 2], [1, wd]])
                    nc.tensor.matmul(v_ps[:, 0:wd], pair_lhs(blob8_t, ob8[s], p),
                                     rhs,
                                     start=(p == 0), stop=(p == BBp[s] - 1),
                                     perf_mode=DR)
                c_t = work.tile([P, W3], f32, tag="c")
                if C_MODE[j] == "sqrt":
                    u_t = work.tile([P, W3], f32, tag="u")
                    nc.scalar.activation(u_t[:, 0:wd], v_ps[:, 0:wd], AF.Abs)
                    nc.scalar.activation(c_t[:, 0:wd], u_t[:, 0:wd], AF.Sqrt,
                                         scale=b4_ap)
                    nc.scalar.sqrt(c_t[:, 0:wd], c_t[:, 0:wd])
                else:
                    t1 = work.tile([P, W3], i32, tag="u")
                    nc.vector.tensor_scalar(
                        out=t1[:, 0:wd], in0=v_ps[:, 0:wd].bitcast(i32),
                        scalar1=0x7FFFFFFF, op0=ALU.bitwise_and,
                        scalar2=2, op1=ALU.logical_shift_right)
                    nc.scalar.activation(c_t[:, 0:wd].bitcast(i32), t1[:, 0:wd],
                                         AF.Identity, bias=k_ap)
                m_t = work.tile([P, W3], f32, tag="m")
                ENG[MT_ENG[j]].scalar_tensor_tensor(
                    out=m_t[:, 0:wd], in0=v_ps[:, 0:wd], scalar=a_ap,
                    in1=c_t[:, 0:wd], op0=ALU.mult, op1=ALU.min,
                )
                w_t = w_bufs[j % len(w_bufs)]
                w_td[j] = w_t
                ENG[WT_ENG[j]].scalar_tensor_tensor(
                    out=w_t[:, PADS:PADS + wd], in0=c_t[:, 0:wd], scalar=-1.0,
                    in1=m_t[:, 0:wd], op0=ALU.mult, op1=ALU.max,
                )
                # re-zero the mid pads the stt overwrote (1 or 2 of them),
                # on the same queue as the stt
                pm = w_t[:, SEC:SEC + 1]
                ENG[WT_ENG[j]].memset(
                    bass.AP(tensor=pm.tensor, offset=pm.offset,
                            ap=[[pm.ap[0][0], P], [SEC, NSEC_H[h] - 1],
                                [1, PADS]]), 0.0)

            def emit_aft(j):
                s, h = CHUNKS[j]
                wd = WH[h]
                o_ps, w_t = o_psd.pop(j), w_td.pop(j)
                for d in range(BA[s]):
                    ws = PADS - d
                    nc.tensor.matmul(o_ps[:, 0:wd], wa_slice(s, d),
                                     w_t[:, ws:ws + wd],
                                     start=False, stop=(d == BA[s] - 1))
                oq = oq_tiles[(j // 3) % 2]
                out_t = oq[:, (j % 3) * W3:(j % 3) * W3 + wd]
                ecopy(OC_ENG[j], out_t, o_ps[:, 0:wd])
                if j == NCH - 2:
                    # ship the last quad's first two chunks early so only a
                    # third of the final transfer sits in the drain
                    nc.sync.dma_start(out=d_out[j // 3][:, :2 * W3],
                                      in_=oq[:, :2 * W3])
                elif j == NCH - 1:
                    nc.sync.dma_start(out=d_out[j // 3][:, 2 * W3:],
                                      in_=oq[:, 2 * W3:])
                elif j % 3 == 2:
                    nc.sync.dma_start(out=d_out[j // 3], in_=oq)

            for t in range(NCH + AFT_LAG):
                if t == 2:
                    emit_me(1)
                if t == 8:
                    emit_me(2)
                if t < NCH:
                    emit_lin(t)
                if 0 <= t - BEF_LAG < NCH:
                    emit_bef_pointwise(t - BEF_LAG)
                if 0 <= t - AFT_LAG < NCH:
                    emit_aft(t - AFT_LAG)
    nc.compile()
    return nc


def _prep(inputs):
    key = "prog"
    if key not in _CACHE:
        meta = _build_host(
            inputs["me_fir"], inputs["lin_fir"], inputs["nlin_fir_before"],
            inputs["nlin_fir_after"], inputs["lpf_lin_b"], inputs["lpf_lin_a"],
            inputs["lpf_nlin_b"], inputs["lpf_nlin_a"],
            np.asarray(inputs["lin_gain"], np.float64),
            np.asarray(inputs["nlin_a"], np.float64),
            np.asarray(inputs["nlin_b"], np.float64),
        )
        _CACHE[key] = (meta, _build_program(meta))
    return _CACHE[key]


def _in_maps(meta, x):
    xf = _fold_x(np.asarray(x, np.float32)).reshape(P, -1)
    XSEC = PADX + R
    head = np.ascontiguousarray(
        np.concatenate([meta["wme"], xf[:, :3 * XSEC]], axis=1))
    xfr = np.ascontiguousarray(xf[:, 3 * XSEC:])
    return [
        {"head": head, "xfr": xfr,
         "blobA": meta["blobA"][c].astype(BF16),
         "blobB": meta["blobB"][c].astype(BF16),
         "blob8": meta["blob8"][c],
         "scal": np.ascontiguousarray(np.broadcast_to(meta["scal"][c], (P, N_SLOTS * 4)))}
        for c in range(N_CORES)
    ]


def _decode(meta, youts):
    PADS = meta["PADS"]
    SEC = PADS + R
    WW = SEC + R
    W3 = 2 * SEC + R
    slot_ch = meta["slot_ch"]
    out = np.zeros((B, F, T), np.float32)
    for c in range(N_CORES):
        yo = np.asarray(youts[c], dtype=np.float32)
        for j, (s, h) in enumerate(CHUNKS):
            if s == 6 and c >= 2:
                continue
            f = slot_ch[c, s]
            inv = 1.0 / meta["gam"][c, s]
            for bi in range(NSEC_H[h]):
                b = QH[h] + bi
                col = (j % 3) * W3 + bi * SEC
                out[b, f, :] = yo[j // 3, :, col:col + R].T.reshape(R * P)[:T] * inv
    return out


def kernel(**inputs):
    meta, nc = _prep(inputs)
    from concourse.bass_utils import run_bass_kernel_spmd

    res = run_bass_kernel_spmd(nc, _in_maps(meta, inputs["x"]),
                               core_ids=list(range(N_CORES)),
                               trace=bool(inputs.get("_trace", False)))
    out = _decode(meta, [res.results[c]["yout"] for c in range(N_CORES)])
    if inputs.get("_return_res", False):
        return out, res
    return out


# revision 106
# speedup vs baseline: 1.0645x; 1.0180x over previous
"""DRNL filterbank Trainium2 kernel, v5.

Banded-Toeplitz formulation (ME/LIN/AFT bf16 band matmuls, BEF fp8
DoubleRow pairs, broken-stick pointwise on vector+scalar).

v5: 3-section chunks. A matmul's psum output must fit one 2KB bank
(512 f32) and the 3-section width 2*SEC+R = 491 fits exactly, so each
LIN/BEF/AFT band runs once per 3 batches instead of once per 2
(28 -> 21 chunks, 264 -> 198 matmuls/core) while keeping the original
1-bank psum rotation (o x4, v x3, me). 4-section (658) fails the ISA
check. The ~140ns fixed per-matmul cost makes fewer-but-wider strictly
better. ME also runs in 3-batch groups aligned with the chunk groups.
Startup: each DMA costs ~1.3us of DMA-engine pipeline latency, so the
whole first-matmul critical path (wme + ME group 0's xf slice) ships
as ONE "head" DMA on the SP queue; the Act queue streams blob8/blobB
in parallel on its own DMA engine. Do NOT shrink the work pool below
bufs=4 (bufs=2 miscomputes), do NOT truncate the ME FIR (1e-3 trunc
-> 1.7e-2 rel err; per-channel LIN gains amplify it), and do NOT put
sqrt-mode on the light tail chunks (serial 3-act latency stalls AFT).

v4 vs v3:
  - DMA consolidation: weight blobs packed in first-use order (bf16
    LIN/AFT blobs + fp8 BEF blob, blobB split in two), one contiguous
    xf DMA (k-major host fold), host-replicated scal, quad output DMAs.
    54 -> 14 DMA_DIRECT2D issues across both HWDGE queues (SP carries
    wme/xf/scal + outputs, Activation carries the weight blobs so the
    first-matmul path skips the ACT_TABLE_LOAD stall).
  - Truncation tolerances raised to 4e-3 / 8e-2 (292 -> 264 matmuls):
    the output error is floored by the final bf16 output-copy
    quantization at the response peak (~4e-3 rel), so shorter IR tails
    are free until truncation error approaches that floor.
  - fp8 DoubleRow for AFT was tried and reverted: pair matmuls measure
    ~450ns vs ~280ns bf16 singles (DoubleRow disables FWL and the dual
    rhs stream is fetch-bound), so the win never materializes, and the
    required one-column-shifted fp8 w copy stalls the PE. The PE also
    rejects small pair-dim rhs strides at runtime (dup_off=-1 aborts),
    so a second shifted signal copy is mandatory for any pair rhs.
Sharding: channels across 8 cores, slot-structured SPMD.
"""
import numpy as np
import ml_dtypes

P = 128
B, T, F = 8, 20000, 50
R = (T + P - 1) // P
N_CORES = 8
N_SLOTS = 7
TRUNC_LIN = 4e-3
TRUNC_NL = 8e-2
IR_LEN = 4096
BME = 5
PADX = 4

_CACHE = {}

BF16 = ml_dtypes.bfloat16
FP8 = ml_dtypes.float8_e4m3fn

# 3-section chunks: a matmul's psum output must stay within one 2KB bank
# (512 f32), and 2*SEC+WW = 491 fits exactly -- so each chunk covers 3
# batches (the last group covers the 2 remaining). 21 chunks total.
CHUNK_SLOTS = [4, 0, 6, 1, 5, 2, 3]
_GROUP_ORDERS = {0: [4, 0, 6, 1, 5, 2, 3], 1: [0, 6, 1, 5, 2, 4, 3],
                 2: [0, 6, 1, 5, 2, 3, 4]}
CHUNKS = [(s, h) for h in (0, 1, 2) for s in _GROUP_ORDERS[h]]
NCH = len(CHUNKS)
NSEC_H = [3, 3, 2]   # batches (sections groups) per h
QH = [0, 3, 6]       # first batch of each group
# pointwise engine schedule (tuned against the trace):
C_MODE = ["int" if (j * 14) % 21 < 14 else "sqrt" for j in range(NCH)]
MT_ENG = ["vector"] * NCH           # STT min(a*v, c)   (PSUM read -> vector only)
WT_ENG = ["vector"] * NCH           # STT w = max(-c, m) (Pool rejects STT entirely)
OC_ENG = ["scalar"] * NCH           # psum -> bf16 output copy
XC_ENG = ["gpsimd", "gpsimd"]       # xme8 main/shifted cast engines
BEF_LAG = 1   # BEF_j issues one chunk-slot after LIN_j
AFT_LAG = 3   # AFT_j issues in iteration j+AFT_LAG
BLOBA_SLOTS = CHUNK_SLOTS[:2]
# blob B packed in first-use order (LIN at slot's first chunk, AFT 3 chunks
# later), split into two DMAs after the 5th block so early consumers don't
# wait on the full transfer
BLOBB_ORDER = [("lin", 6), ("lin", 1), ("aft", 4), ("lin", 5), ("aft", 0),
               ("lin", 2), ("aft", 6), ("lin", 3), ("aft", 1), ("aft", 5),
               ("aft", 2), ("aft", 3)]
BLOBB_SPLIT = 5


def _lfilter_vec(x, b, a):
    b0, b1, b2 = b[:, 0], b[:, 1], b[:, 2]
    a1, a2 = a[:, 1], a[:, 2]
    y = np.zeros_like(x)
    z1 = np.zeros(x.shape[0])
    z2 = np.zeros(x.shape[0])
    for t in range(x.shape[-1]):
        xt = x[:, t]
        yt = b0 * xt + z1
        z1 = b1 * xt - a1 * yt + z2
        z2 = b2 * xt - a2 * yt
        y[:, t] = yt
    return y


def _cascade_ir(b, a, n, times):
    h = np.zeros((b.shape[0], n))
    h[:, 0] = 1.0
    for _ in range(times):
        h = _lfilter_vec(h, b, a)
    return h


def _trunc(h, tol):
    m = np.abs(h).max()
    idx = np.nonzero(np.abs(h) > tol * m)[0]
    return h[: int(idx[-1]) + 1] if len(idx) else h[:1]


def _nb(h):
    return (len(h) + P - 2) // P + 1


def _toeplitz_band(h, d):
    k = np.arange(P)[:, None]
    i = np.arange(P)[None, :]
    idx = P * d + i - k
    ok = (idx >= 0) & (idx < len(h))
    return np.where(ok, np.asarray(h, np.float64)[np.clip(idx, 0, len(h) - 1)], 0.0)


def _bands_cat(h, nb, dtype):
    W = np.concatenate([_toeplitz_band(h, d) for d in range(nb)], axis=1)
    return np.ascontiguousarray(W).astype(dtype)


def _pairs_cat(h, nbp, dtype):
    mats = []
    for p in range(nbp):
        mats.append(_toeplitz_band(h, 2 * p))
        mats.append(_toeplitz_band(h, 2 * p + 1))
    W = np.concatenate(mats, axis=1)
    return np.ascontiguousarray(W).astype(dtype)


def _tune_k(b):
    """Best int32 bias K: bitcast((bits(x)>>2)+K) ~= b*x**0.25 over x range."""
    x = np.float32(np.logspace(-6, 1.5, 4000))
    t1 = (x.view(np.int32) & 0x7FFFFFFF) >> 2
    ref = np.float64(b) * np.float64(x) ** 0.25
    b0 = 127 << 23
    base = np.int64(np.float32(b).view(np.int32)) - b0 // 4
    best = (1e9, 0)
    for sig in np.linspace(-0.02, 0.10, 121):
        K = np.int64(base - int(sig * (1 << 23)))
        c = (t1.astype(np.int64) + K).astype(np.int32).view(np.float32)
        e = np.abs(c - ref) / ref
        best = min(best, (float(e.max()), int(K)))
    return best[1]


def _build_host(me_fir, lin_fir, nlin_fir_before, nlin_fir_after,
                lpf_lin_b, lpf_lin_a, lpf_nlin_b, lpf_nlin_a,
                lin_gain, nlin_a, nlin_b):
    ir4 = _cascade_ir(lpf_lin_b.astype(np.float64), lpf_lin_a.astype(np.float64), IR_LEN, 4)
    ir3 = _cascade_ir(lpf_nlin_b.astype(np.float64), lpf_nlin_a.astype(np.float64), IR_LEN, 3)

    scale = 10.0 ** ((93.98 - 100.0) / 20.0)
    ME = np.asarray(me_fir, np.float64) * scale
    bme = _nb(ME)

    LIN, BEF, AFT = [], [], []
    for f in range(F):
        LIN.append(_trunc(lin_gain[f] * np.convolve(np.asarray(lin_fir[f], np.float64), ir4[f]), TRUNC_LIN))
        BEF.append(_trunc(np.asarray(nlin_fir_before[f], np.float64), TRUNC_NL))
        AFT.append(_trunc(np.convolve(np.asarray(nlin_fir_after[f], np.float64), ir3[f]), TRUNC_NL))

    nbp = lambda h: (_nb(h) + 1) // 2
    cost = [_nb(LIN[f]) + nbp(BEF[f]) + _nb(AFT[f]) for f in range(F)]
    order = np.argsort(-np.asarray(cost), kind="stable")

    slot_ch = np.zeros((N_CORES, N_SLOTS), np.int64)
    for s in range(6):
        for c in range(N_CORES):
            slot_ch[c, s] = order[8 * s + c]
    for c in range(N_CORES):
        slot_ch[c, 6] = order[48 + (c % 2)]

    BL = [max(_nb(LIN[slot_ch[c, s]]) for c in range(N_CORES)) for s in range(N_SLOTS)]
    BBp = [max(nbp(BEF[slot_ch[c, s]]) for c in range(N_CORES)) for s in range(N_SLOTS)]
    BA = [max(_nb(AFT[slot_ch[c, s]]) for c in range(N_CORES)) for s in range(N_SLOTS)]

    sh = max(max(BL) - 1, 2 * max(BBp) - 1, max(BA) - 1)
    PADS = (sh + 1 + 1) // 2 * 2

    gam = np.ones((N_CORES, N_SLOTS), np.float64)

    wme = _bands_cat(ME, bme, BF16)
    blobA, blobB, blob8 = [], [], []
    for c in range(N_CORES):
        a16 = [_bands_cat(gam[c, s] * LIN[slot_ch[c, s]], BL[s], BF16)
               for s in BLOBA_SLOTS]
        b16 = []
        for kind, s in BLOBB_ORDER:
            h = (gam[c, s] * LIN[slot_ch[c, s]] if kind == "lin"
                 else gam[c, s] * AFT[slot_ch[c, s]])
            b16.append(_bands_cat(h, BL[s] if kind == "lin" else BA[s], BF16))
        p8 = [_pairs_cat(BEF[slot_ch[c, s]], BBp[s], FP8) for s in CHUNK_SLOTS]
        blobA.append(np.concatenate(a16, axis=1))
        blobB.append(np.concatenate(b16, axis=1))
        blob8.append(np.concatenate(p8, axis=1))

    scal = np.zeros((N_CORES, N_SLOTS * 4), np.float32)
    for c in range(N_CORES):
        for s in range(N_SLOTS):
            f = slot_ch[c, s]
            scal[c, 4 * s + 0] = nlin_a[f]
            scal[c, 4 * s + 1] = float(nlin_b[f]) ** 4
            scal[c].view(np.int32)[4 * s + 2] = _tune_k(float(nlin_b[f]))

    return {
        "slot_ch": slot_ch, "BL": BL, "BBp": BBp, "BA": BA, "PADS": PADS,
        "gam": gam, "BME": bme,
        "wme": wme, "blobA": blobA, "blobB": blobB, "blob8": blob8,
        "scal": scal,
    }


def _fold_x(x):
    # k-major layout [P, B, XSEC]: the xf DMA is 128 contiguous rows
    xp = np.zeros((B, R * P), np.float32)
    xp[:, :T] = x
    xf = np.zeros((P, B, PADX + R), np.float32)
    xf[:, :, PADX:] = xp.reshape(B, R, P).transpose(2, 0, 1)
    return np.ascontiguousarray(xf).astype(BF16)


def _build_program(meta):
    import concourse.bacc as bacc
    import concourse.bass as bass
    from concourse import mybir
    from concourse.tile import TileContext

    BL, BBp, BA, PADS = meta["BL"], meta["BBp"], meta["BA"], meta["PADS"]
    BME = meta["BME"]
    SEC = PADS + R
    WW = SEC + R
    W3 = 2 * SEC + R     # 3-section chunk width (491 f32: fits one psum bank)
    WH = [W3, W3, WW]    # matmul/pointwise width per group h
    XSEC = PADX + R
    XW = XSEC + R
    f32, bf16, f8 = mybir.dt.float32, mybir.dt.bfloat16, mybir.dt.float8e4
    i32 = mybir.dt.int32
    AF = mybir.ActivationFunctionType
    ALU = mybir.AluOpType
    DR = mybir.MatmulPerfMode.DoubleRow

    olA, olB, ob8, oa = {}, {}, {}, {}
    off = 0
    for s in BLOBA_SLOTS:
        olA[s] = off
        off += BL[s]
    nA = off
    off = 0
    nB_split = 0
    for bi, (kind, s) in enumerate(BLOBB_ORDER):
        if kind == "lin":
            olB[s] = off
            off += BL[s]
        else:
            oa[s] = off
            off += BA[s]
        if bi + 1 == BLOBB_SPLIT:
            nB_split = off
    nB = off
    off = 0
    for s in CHUNK_SLOTS:
        ob8[s] = off
        off += BBp[s]
    n8 = off

    HEADW = BME * P + 3 * XSEC
    nc = bacc.Bacc("TRN2", target_bir_lowering=False, debug=False, num_devices=N_CORES)
    # head = wme bands + ME group 0's xf slice: the whole first-matmul
    # critical path arrives with a single DMA (each extra DMA adds ~1.3us
    # of DMA-engine pipeline latency)
    d_head = nc.dram_tensor("head", [P, HEADW], bf16, kind="ExternalInput").ap()
    d_xfr = nc.dram_tensor("xfr", [P, 5 * XSEC], bf16, kind="ExternalInput").ap()
    d_blobA = nc.dram_tensor("blobA", [P, nA * P], bf16, kind="ExternalInput").ap()
    d_blobB = nc.dram_tensor("blobB", [P, nB * P], bf16, kind="ExternalInput").ap()
    d_blob8 = nc.dram_tensor("blob8", [P, n8 * 2 * P], f8, kind="ExternalInput").ap()
    d_scal = nc.dram_tensor("scal", [P, N_SLOTS * 4], f32, kind="ExternalInput").ap()
    d_out = nc.dram_tensor("yout", [7, P, 3 * W3], bf16, kind="ExternalOutput").ap()

    def pair_rhs(tile, col_off, dup_off):
        base = tile[:, 0:1]
        return bass.AP(tensor=base.tensor, offset=base.offset + col_off,
                       ap=[[base.ap[0][0], P], [dup_off, 2], [1, WW]])

    def pair_lhs(tile, band_off, p):
        s = tile[:, (band_off + p) * 2 * P:(band_off + p + 1) * 2 * P]
        return bass.AP(tensor=s.tensor, offset=s.offset,
                       ap=[[s.ap[0][0], P], [P, 2], [1, P]])

    ENG = None  # set inside context

    with TileContext(nc) as tc:
        with (
            tc.tile_pool(name="singles", bufs=1) as singles,
            tc.tile_pool(name="work", bufs=4) as work,
            tc.tile_pool(name="ps", bufs=1, space="PSUM") as ps,
        ):
            ENG = {"vector": nc.vector, "scalar": nc.scalar, "gpsimd": nc.gpsimd}

            def ecopy(eng, out, in_):
                if eng == "scalar":
                    nc.scalar.activation(out, in_, AF.Copy)
                else:
                    ENG[eng].tensor_copy(out=out, in_=in_)
            # SP: head (first-matmul critical path), blobA (LIN chunk 0),
            # xf rest. Act: scal, then the remaining weight blobs — the two
            # queues transfer on separate DMA engines.
            head_t = singles.tile([P, HEADW], bf16)
            nc.sync.dma_start(out=head_t, in_=d_head)
            blobA_t = singles.tile([P, nA * P], bf16)
            nc.sync.dma_start(out=blobA_t, in_=d_blobA)
            xfr_t = singles.tile([P, 5 * XSEC], bf16)
            nc.sync.dma_start(out=xfr_t, in_=d_xfr)
            blob8_t = singles.tile([P, n8 * 2 * P], f8)
            nc.scalar.dma_start(out=blob8_t, in_=d_blob8)
            scal_t = singles.tile([P, N_SLOTS * 4], f32)
            nc.scalar.dma_start(out=scal_t, in_=d_scal)
            blobB_t = singles.tile([P, nB * P], bf16)
            nc.scalar.dma_start(out=blobB_t[:, :nB_split * P],
                                in_=d_blobB[:, :nB_split * P])
            nc.scalar.dma_start(out=blobB_t[:, nB_split * P:],
                                in_=d_blobB[:, nB_split * P:])

            def wl_slice(s, d):
                if s in olA:
                    return blobA_t[:, (olA[s] + d) * P:(olA[s] + d + 1) * P]
                return blobB_t[:, (olB[s] + d) * P:(olB[s] + d + 1) * P]

            def wa_slice(s, d):
                return blobB_t[:, (oa[s] + d) * P:(oa[s] + d + 1) * P]

            O8 = (B * SEC + 7) // 4 * 4
            xme16 = singles.tile([P, B * SEC], bf16)
            xme8 = singles.tile([P, O8 + B * SEC + 4], f8)
            for qq in range(B):
                nc.vector.memset(xme16[:, qq * SEC:qq * SEC + PADS], 0.0)
                nc.gpsimd.memset(xme8[:, qq * SEC:qq * SEC + PADS], 0.0)
                lo = O8 + qq * SEC + (1 if qq else 0)
                nc.gpsimd.memset(xme8[:, lo:O8 + qq * SEC + PADS + 1], 0.0)

            # ME in 3-batch groups aligned with the chunk groups: group h
            # produces exactly the xme sections chunk group h consumes
            XW3 = 2 * XSEC + R

            def emit_me(g):
                q = QH[g]
                n = NSEC_H[g]
                xw = (n - 1) * XSEC + R
                mp = ps.tile([P, XW3], f32, tag="me")
                for d in range(BME):
                    if g == 0:
                        ws = BME * P + q * XSEC + PADX - d
                        rhs = head_t[:, ws:ws + xw]
                    else:
                        ws = (q - 3) * XSEC + PADX - d
                        rhs = xfr_t[:, ws:ws + xw]
                    nc.tensor.matmul(mp[:, 0:xw], head_t[:, d * P:(d + 1) * P],
                                     rhs, start=(d == 0), stop=(d == BME - 1))
                for bi in range(n):
                    qq = q + bi
                    # spread the psum->xme16 copies across vector+scalar so
                    # the chunk's LIN (which needs all n sections) starts
                    # ~0.4us sooner than with a serial vector chain
                    ecopy("scalar" if bi == 1 else "vector",
                          xme16[:, qq * SEC + PADS:(qq + 1) * SEC],
                          mp[:, bi * XSEC:bi * XSEC + R])
                    src = xme16[:, qq * SEC + PADS:(qq + 1) * SEC]
                    ecopy(XC_ENG[0], xme8[:, qq * SEC + PADS:(qq + 1) * SEC], src)
                    ecopy(XC_ENG[1],
                          xme8[:, O8 + qq * SEC + PADS + 1:O8 + (qq + 1) * SEC + 1],
                          src)

            emit_me(0)

            o_psd, v_psd, w_td = {}, {}, {}
            w_bufs = []
            for i in range(AFT_LAG + 1):
                wbuf = singles.tile([P, 3 * SEC], bf16, tag=f"wbuf{i}")
                nc.vector.memset(wbuf[:, 0:PADS], 0.0)
                w_bufs.append(wbuf)
            oq_tiles = []
            for i in range(2):
                oq_t = singles.tile([P, 3 * W3], bf16, tag=f"oq{i}")
                oq_tiles.append(oq_t)

            def emit_lin(j):
                s, h = CHUNKS[j]
                q = QH[h]
                o_ps = ps.tile([P, W3], f32, tag=f"o{j % 4}")
                o_psd[j] = o_ps
                for d in range(BL[s]):
                    ws = q * SEC + PADS - d
                    nc.tensor.matmul(o_ps[:, 0:WH[h]], wl_slice(s, d),
                                     xme16[:, ws:ws + WH[h]],
                                     start=(d == 0), stop=False)

            def emit_bef_pointwise(j):
                s, h = CHUNKS[j]
                q = QH[h]
                wd = WH[h]
                a_ap = scal_t[:, 4 * s + 0:4 * s + 1]
                b4_ap = scal_t[:, 4 * s + 1:4 * s + 2]
                k_ap = scal_t.bitcast(i32)[:, 4 * s + 2:4 * s + 3]
                v_ps = ps.tile([P, W3], f32, tag=f"v{j % 3}")
                v_psd[j] = v_ps
                for p in range(BBp[s]):
                    base = xme8[:, 0:1]
                    rhs = bass.AP(tensor=base.tensor,
                                  offset=base.offset + q * SEC + PADS - 2 * p,
                                  ap=[[base.ap[0][0], P], [O8, 2], [1, wd]])
                    nc.tensor.matmul(v_ps[:, 0:wd], pair_lhs(blob8_t, ob8[s], p),
                                     rhs,
                                     start=(p == 0), stop=(p == BBp[s] - 1),
                                     perf_mode=DR)
                c_t = work.tile([P, W3], f32, tag="c")
                if C_MODE[j] == "sqrt":
                    u_t = work.tile([P, W3], f32, tag="u")
                    nc.scalar.activation(u_t[:, 0:wd], v_ps[:, 0:wd], AF.Abs)
                    nc.scalar.activation(c_t[:, 0:wd], u_t[:, 0:wd], AF.Sqrt,
                                         scale=b4_ap)
                    nc.scalar.sqrt(c_t[:, 0:wd], c_t[:, 0:wd])
                else:
                    t1 = work.tile([P, W3], i32, tag="u")
                    nc.vector.tensor_scalar(
                        out=t1[:, 0:wd], in0=v_ps[:, 0:wd].bitcast(i32),
                        scalar1=0x7FFFFFFF, op0=ALU.bitwise_and,
                        scalar2=2, op1=ALU.logical_shift_right)
                    nc.scalar.activation(c_t[:, 0:wd].bitcast(i32), t1[:, 0:wd],
                                         AF.Identity, bias=k_ap)
                m_t = work.tile([P, W3], f32, tag="m")
                ENG[MT_ENG[j]].scalar_tensor_tensor(
                    out=m_t[:, 0:wd], in0=v_ps[:, 0:wd], scalar=a_ap,
                    in1=c_t[:, 0:wd], op0=ALU.mult, op1=ALU.min,
                )
                w_t = w_bufs[j % len(w_bufs)]
                w_td[j] = w_t
                ENG[WT_ENG[j]].scalar_tensor_tensor(
                    out=w_t[:, PADS:PADS + wd], in0=c_t[:, 0:wd], scalar=-1.0,
                    in1=m_t[:, 0:wd], op0=ALU.mult, op1=ALU.max,
                )
                # re-zero the mid pads the stt overwrote (1 or 2 of them),
                # on the same queue as the stt
                pm = w_t[:, SEC:SEC + 1]
                ENG[WT_ENG[j]].memset(
                    bass.AP(tensor=pm.tensor, offset=pm.offset,
                            ap=[[pm.ap[0][0], P], [SEC, NSEC_H[h] - 1],
                                [1, PADS]]), 0.0)

            def emit_aft(j):
                s, h = CHUNKS[j]
                wd = WH[h]
                o_ps, w_t = o_psd.pop(j), w_td.pop(j)
                for d in range(BA[s]):
                    ws = PADS - d
                    nc.tensor.matmul(o_ps[:, 0:wd], wa_slice(s, d),
                                     w_t[:, ws:ws + wd],
                                     start=False, stop=(d == BA[s] - 1))
                oq = oq_tiles[(j // 3) % 2]
                out_t = oq[:, (j % 3) * W3:(j % 3) * W3 + wd]
                ecopy(OC_ENG[j], out_t, o_ps[:, 0:wd])
                if j == NCH - 2:
                    # ship the last quad's first two chunks early so only a
                    # third of the final transfer sits in the drain
                    nc.sync.dma_start(out=d_out[j // 3][:, :2 * W3],
                                      in_=oq[:, :2 * W3])
                elif j == NCH - 1:
                    nc.sync.dma_start(out=d_out[j // 3][:, 2 * W3:],
                                      in_=oq[:, 2 * W3:])
                elif j % 3 == 2:
                    nc.sync.dma_start(out=d_out[j // 3], in_=oq)

            for t in range(NCH + AFT_LAG):
                if t == 2:
                    emit_me(1)
                if t == 8:
                    emit_me(2)
                if t < NCH:
                    emit_lin(t)
                if 0 <= t - BEF_LAG < NCH:
                    emit_bef_pointwise(t - BEF_LAG)
                if 0 <= t - AFT_LAG < NCH:
                    emit_aft(t - AFT_LAG)
    nc.compile()
    return nc


def _prep(inputs):
    key = "prog"
    if key not in _CACHE:
        meta = _build_host(
            inputs["me_fir"], inputs["lin_fir"], inputs["nlin_fir_before"],
            inputs["nlin_fir_after"], inputs["lpf_lin_b"], inputs["lpf_lin_a"],
            inputs["lpf_nlin_b"], inputs["lpf_nlin_a"],
            np.asarray(inputs["lin_gain"], np.float64),
            np.asarray(inputs["nlin_a"], np.float64),
            np.asarray(inputs["nlin_b"], np.float64),
        )
        _CACHE[key] = (meta, _build_program(meta))
    return _CACHE[key]


def _in_maps(meta, x):
    xf = _fold_x(np.asarray(x, np.float32)).reshape(P, -1)
    XSEC = PADX + R
    head = np.ascontiguousarray(
        np.concatenate([meta["wme"], xf[:, :3 * XSEC]], axis=1))
    xfr = np.ascontiguousarray(xf[:, 3 * XSEC:])
    return [
        {"head": head, "xfr": xfr,
         "blobA": meta["blobA"][c].astype(BF16),
         "blobB": meta["blobB"][c].astype(BF16),
         "blob8": meta["blob8"][c],
         "scal": np.ascontiguousarray(np.broadcast_to(meta["scal"][c], (P, N_SLOTS * 4)))}
        for c in range(N_CORES)
    ]


def _decode(meta, youts):
    PADS = meta["PADS"]
    SEC = PADS + R
    WW = SEC + R
    W3 = 2 * SEC + R
    slot_ch = meta["slot_ch"]
    out = np.zeros((B, F, T), np.float32)
    for c in range(N_CORES):
        yo = np.asarray(youts[c], dtype=np.float32)
        for j, (s, h) in enumerate(CHUNKS):
            if s == 6 and c >= 2:
                continue
            f = slot_ch[c, s]
            inv = 1.0 / meta["gam"][c, s]
            for bi in range(NSEC_H[h]):
                b = QH[h] + bi
                col = (j % 3) * W3 + bi * SEC
                out[b, f, :] = yo[j // 3, :, col:col + R].T.reshape(R * P)[:T] * inv
    return out


def kernel(**inputs):
    meta, nc = _prep(inputs)
    from concourse.bass_utils import run_bass_kernel_spmd

    res = run_bass_kernel_spmd(nc, _in_maps(meta, inputs["x"]),
                               core_ids=list(range(N_CORES)),
                               trace=bool(inputs.get("_trace", False)))
    out = _decode(meta, [res.results[c]["yout"] for c in range(N_CORES)])
    if inputs.get("_return_res", False):
        return out, res
    return out
